# revision 3
# baseline (speedup 1.0000x reference)
"""Trainium2 Bass kernel for GNN message passing (nn_Actor_71141838291282).

Algorithm (per message-passing iteration, T=8):
    msg  = selu(ls[first] @ Wm1 + ls[second] @ Wm2 + bm)     [M, 32]
    agg  = segment_sum(msg, second, E)                        [E, 32]
    ls   = GRU(agg, ls)                                       [E, 32]
Readout: graph segment-sum + 3-layer MLP (done host-side; negligible work).

Distribution: 8 NeuronCores, shard by DESTINATION node (states_second).
Core c owns dests [c*EL, (c+1)*EL), EL = E/8 = 32768.
Per iteration each core:
  1. A = ls_loc @ Wm1 + bm (fp16, node-major) -> DRAM;  B = ls_loc @ Wm2 (fp16, SBUF)
  2. AllGather A -> full table A_ext [E, 32] fp16 in local DRAM
  3. "Rounds": dests per core are relabeled by descending in-degree (host-side
     permutation), so round r = the r-th edge of every dest with degree > r is a
     contiguous PREFIX of the dest space.  For each round: prefill a buffer with
     B[dest], indirect-DMA gather-add A_ext[first] (cce add) into it, apply SELU
     (exp(min(x,0)) composite, no selects), accumulate into agg (f32).
     Pad slots gather a -3e4 pad row => selu == -lam*alpha exactly; corrected by
     initializing agg with +lam*alpha*npad (static).
  4. GRU feat-major via PE matmuls (W/U stationary) + DVE/ACT elementwise.

All indices/permutations are computed host-side in numpy (static data).
"""

import math
import numpy as np

import concourse.bass as bass
import concourse.mybir as mybir
import concourse.tile as tile
from concourse import bacc
from concourse.bass_utils import run_bass_kernel_spmd

F32 = mybir.dt.float32
F16 = mybir.dt.float16
I32 = mybir.dt.int32

LAM = 1.0507009873554805
ALPHA = 1.6732632423543772
PAD_VAL = -30000.0  # pad row value in A table (f16 range)

P = 128


class Cfg:
    def __init__(self, E=262144, T=8, n_cores=8):
        self.E = E
        self.T = T
        self.NC = n_cores
        self.EL = E // n_cores
        self.C = self.EL // P            # grid cols (dest rank i at (i%P, i//P))
        assert self.EL % P == 0


# ---------------------------------------------------------------------------
# Host preprocessing
# ---------------------------------------------------------------------------

def preprocess(cfg, states_first, states_second):
    """Build per-core static index data.

    Returns dict with:
      order[c]   : [EL]   local node id for dest rank i  (descending degree)
      gperm      : [E]    global A-table row for global node u
      idx[c]     : [P, CTOT] int32 gather rows per round (concatenated cols)
      corr[c]    : [P, C] f32  = +LAM*ALPHA * npad  (agg init)
      C_r        : list of per-round col counts (compile-time consts)
    """
    E, EL, NC, C = cfg.E, cfg.EL, cfg.NC, cfg.C
    first = np.asarray(states_first, dtype=np.int64)
    second = np.asarray(states_second, dtype=np.int64)

    core_of = second // EL
    orders = []
    ranks = np.empty(E, dtype=np.int64)
    degs_sorted = []
    per_core_edges = []
    for c in range(NC):
        m = core_of == c
        ef = first[m]
        es = second[m] - c * EL
        deg = np.bincount(es, minlength=EL)
        order = np.argsort(-deg, kind="stable")        # rank -> local id
        rank = np.empty(EL, dtype=np.int64)
        rank[order] = np.arange(EL)
        orders.append(order)
        ranks[c * EL:(c + 1) * EL] = rank
        degs_sorted.append(deg[order])                  # descending
        per_core_edges.append((ef, es, rank))

    # global A-table row of node u (per-core perm layout, rank-ordered chunks)
    gperm = (np.arange(E) // EL) * EL + ranks

    maxdeg = max(int(d[0]) for d in degs_sorted)
    R = maxdeg
    # per-round edge counts n_r per core; global padded col counts
    C_r = []
    for r in range(R):
        n_r_max = max(int(np.count_nonzero(d > r)) for d in degs_sorted)
        C_r.append(max(1, math.ceil(n_r_max / P)))
    CTOT = sum(C_r)

    idx_all = []
    corr_all = []
    for c in range(NC):
        ef, es, rank = per_core_edges[c]
        d_sorted = degs_sorted[c]
        # sort edges by (dest rank, arbitrary); round index = occurrence count
        dest_rank = rank[es]
        o = np.argsort(dest_rank, kind="stable")
        dr = dest_rank[o]
        rows = gperm[ef[o]]
        # occurrence number within each dest
        occ = np.arange(len(dr)) - np.concatenate(
            ([0], np.cumsum(np.bincount(dr, minlength=EL))))[dr]
        idx = np.full((R, EL), cfg.E, dtype=np.int64)   # pad row = E
        idx[occ, dr] = rows
        # build [P, CTOT] layout: round r slots i in [0, P*C_r[r]), slot i=(p+P*cc)
        cols = np.zeros((P, CTOT), dtype=np.int32)
        off = 0
        npad = np.zeros(EL, dtype=np.int64)
        for r in range(R):
            ncols = C_r[r]
            sl = idx[r, :P * ncols]                       # slot i -> row
            pads = sl == cfg.E
            npad[:P * ncols] += pads
            cols[:, off:off + ncols] = sl.reshape(ncols, P).T
            off += ncols
        idx_all.append(cols)
        corr = (LAM * ALPHA) * npad.astype(np.float32)
        corr_all.append(corr.reshape(C, P).T.copy())     # [P, C]

    return dict(orders=orders, gperm=gperm, idx=idx_all, corr=corr_all,
                C_r=C_r, R=R, CTOT=CTOT)


# ---------------------------------------------------------------------------
# Device kernel builder
# ---------------------------------------------------------------------------

def build_full(cfg, C_r, CTOT):
    """Build the complete SPMD graph (all 8 cores run this identically)."""
    E, EL, NC, C, T = cfg.E, cfg.EL, cfg.NC, cfg.C, cfg.T
    R = len(C_r)
    nc = bacc.Bacc("TRN2", target_bir_lowering=False, debug=False,
                   num_devices=NC)

    ls0 = nc.dram_tensor("ls0", [32, EL], F32, kind="ExternalInput")
    idx_in = nc.dram_tensor("idx", [P, CTOT], I32, kind="ExternalInput")
    corr_in = nc.dram_tensor("corr", [P, C], F32, kind="ExternalInput")
    # packed weights: wm1[0:32,0:32] wm2[0:32,32:64] bm[0:32,64] w[0:32,65:161]
    # u[0:32,161:257] b0[0:96,257] b1[0:96,258]
    wp_in = nc.dram_tensor("wpack", [96, 260], F32, kind="ExternalInput")
    ls_out = nc.dram_tensor("ls_out", [32, EL], F32, kind="ExternalOutput")

    a_loc = nc.dram_tensor("a_loc", [EL, 32], F16)
    a_ext = nc.dram_tensor("a_ext", [E + P, 32], F16, addr_space="Shared")
    ls_ping = nc.dram_tensor("ls_ping", [32, EL], F32)
    ls_pong = nc.dram_tensor("ls_pong", [32, EL], F32)

    CH = min(2048, EL)       # chunk (free dim) for A/B and GRU phases
    NMM = min(512, CH)       # matmul free dim
    n_ch = EL // CH
    GCOL = CH // P           # grid cols per chunk
    KB = CH // 32            # 32-blocks per chunk

    AF = mybir.ActivationFunctionType
    ALU = mybir.AluOpType

    with tile.TileContext(nc) as tc:
        with (
            tc.tile_pool(name="sb", bufs=1) as sb,
            tc.tile_pool(name="io", bufs=2) as io,
            tc.tile_pool(name="ab", bufs=1) as ab,
            tc.tile_pool(name="rp", bufs=2) as rp,
            tc.tile_pool(name="mm", bufs=2, space="PSUM") as pmm,
        ):
            # ---- persistent SBUF ----
            idx_sb = sb.tile([P, CTOT], I32, tag="idx")
            nc.sync.dma_start(idx_sb[:], idx_in[:, :])
            corr_sb = sb.tile([P, C], F32, tag="corr")
            nc.sync.dma_start(corr_sb[:], corr_in[:, :])
            wp = sb.tile([96, 260], F32, tag="wp")
            nc.sync.dma_start(wp[:], wp_in[:, :])
            wm1 = wp[0:32, 0:32]
            wm2 = wp[0:32, 32:64]
            bm_ap = wp[0:32, 64:65]
            w_ap = wp[0:32, 65:161]
            u_ap = wp[0:32, 161:257]
            b0_ap = wp[0:96, 257:258]
            b1_ap = wp[0:96, 258:259]

            b_nm = sb.tile([P, C, 32], F16, tag="b_nm")
            a_st = sb.tile([P, C, 32], F16, tag="a_st")
            agg = sb.tile([P, C, 32], F32, tag="agg")

            padrow = sb.tile([P, 32], F16, tag="padrow")
            nc.vector.memset(padrow[:], PAD_VAL)
            nc.sync.dma_start(a_ext[E:E + P, :], padrow[:])

            def ls_src(t):
                if t == 0:
                    return ls0
                return ls_ping if t % 2 == 1 else ls_pong

            def ls_dst(t):
                if t == cfg.T - 1:
                    return ls_out
                return ls_ping if t % 2 == 0 else ls_pong

            for t in range(cfg.T):
                src = ls_src(t)

                # ================= phase 1: A/B =================
                for ch in range(n_ch):
                    o = ch * CH
                    lsc = io.tile([32, CH], F32, tag="lsio")
                    nc.sync.dma_start(lsc[:], src[:, o:o + CH])
                    pa = pmm.tile([32, CH], F32, tag="mm", space="PSUM")
                    pb = pmm.tile([32, CH], F32, tag="mm", space="PSUM")
                    for k in range(CH // NMM):
                        nc.tensor.matmul(pa[:, k * NMM:(k + 1) * NMM],
                                         wm1, lsc[:, k * NMM:(k + 1) * NMM])
                    for k in range(CH // NMM):
                        nc.tensor.matmul(pb[:, k * NMM:(k + 1) * NMM],
                                         wm2, lsc[:, k * NMM:(k + 1) * NMM])
                    aT = ab.tile([32, CH], F16, tag="aT")
                    bT = ab.tile([32, CH], F16, tag="bT")
                    nc.scalar.activation(aT[:], pa[:], AF.Identity, bias=bm_ap)
                    nc.scalar.activation(bT[:], pb[:], AF.Identity)
                    # 32x32 block transpose + block remap into node-major grid
                    # tmp[q, 32k+f] = srcT[f, 32k+q];  node = o + 32k + q;
                    # grid (p, c) = (32*(k%4)+q, ch*GCOL + k//4)
                    for srcT, dstG in ((aT, a_st), (bT, b_nm)):
                        tT = ab.tile([32, CH], F16, tag="tT")
                        nc.vector.transpose(tT[:], srcT[:])
                        t3 = tT[:].rearrange("q (k f) -> q k f", f=32)
                        for p32 in range(4):
                            nc.vector.tensor_copy(
                                dstG[32 * p32:32 * (p32 + 1),
                                     ch * GCOL:(ch + 1) * GCOL, :],
                                t3[:, p32::4, :])
                # A grid -> DRAM table rows (row i = p + P*c)
                a_loc3 = a_loc[:, :].rearrange("(c p) f -> p c f", p=P)
                nc.sync.dma_start(a_loc3, a_st[:])

                # ================= phase 2: AllGather =================
                nc.gpsimd.collective_compute(
                    "AllGather", ALU.bypass,
                    replica_groups=[list(range(NC))],
                    ins=[a_loc[:, :].opt()],
                    outs=[a_ext[0:E, :].opt()],
                )

                # ================= phase 3: rounds =================
                nc.vector.tensor_copy(
                    agg[:], corr_sb[:, :, None].to_broadcast([P, C, 32]))
                off = 0
                for r in range(R):
                    ncols = C_r[r]
                    for h0 in range(0, ncols, P):
                        cw = min(P, ncols - h0)
                        rb = rp.tile([P, P, 32], F16, tag="rb")
                        rbv = rb[:, 0:cw, :]
                        rbf = rb[:].rearrange("p a b -> p (a b)")
                        nc.vector.tensor_copy(rbv, b_nm[:, h0:h0 + cw, :])
                        # HW indirect DMA: one descriptor per partition, one
                        # row per instruction column (2D offset slices only;
                        # 3D out APs crash the DGE).
                        for cc in range(cw):
                            nc.gpsimd.indirect_dma_start(
                                out=rbf[:, cc * 32:(cc + 1) * 32],
                                out_offset=None,
                                in_=a_ext[:, :],
                                in_offset=bass.IndirectOffsetOnAxis(
                                    ap=idx_sb[:, off + h0 + cc:
                                              off + h0 + cc + 1], axis=0),
                                compute_op=ALU.add,
                            )
                        # selu: m=min(x,0); v=LAM*max(x,0) (in-place rb);
                        # e=exp(m) (in-place); s=LAM*ALPHA*e-LAM*ALPHA (ip);
                        # s+=v (ip); agg+=s
                        mt = rp.tile([P, P, 32], F16, tag="mt")
                        mtv = mt[:, 0:cw, :]
                        nc.vector.tensor_scalar(mtv, rbv, 0.0, None, ALU.min)
                        nc.vector.tensor_scalar(rbv, rbv, 0.0, LAM, ALU.max,
                                                ALU.mult)
                        nc.scalar.activation(mtv, mtv, AF.Exp)
                        nc.vector.tensor_scalar(mtv, mtv, LAM * ALPHA,
                                                -LAM * ALPHA, ALU.mult,
                                                ALU.add)
                        nc.vector.tensor_tensor(mtv, mtv, rbv, ALU.add)
                        av = agg[:, h0:h0 + cw, :]
                        nc.vector.tensor_tensor(av, av, mtv, ALU.add)
                    off += ncols

                # ================= phase 4: GRU =================
                dst = ls_dst(t)
                for ch in range(n_ch):
                    o = ch * CH
                    c0 = ch * GCOL
                    # agg chunk -> feat-major agT
                    agT = io.tile([32, CH], F32, tag="agT")
                    tmp2 = io.tile([P, GCOL * 32], F32, tag="tmp2")
                    nc.vector.transpose(
                        tmp2[:],
                        agg[:, c0:c0 + GCOL, :].rearrange("p a b -> p (a b)"))
                    tmp23 = tmp2[:].rearrange("p (c q) -> p c q", q=32)
                    agT3 = agT[:].rearrange("f (c w) -> f c w", w=P)
                    for p32 in range(4):
                        nc.vector.tensor_copy(
                            agT3[:, :, 32 * p32:32 * (p32 + 1)],
                            tmp23[32 * p32:32 * (p32 + 1), :, :])
                    lsc = io.tile([32, CH], F32, tag="lsio")
                    nc.sync.dma_start(lsc[:], src[:, o:o + CH])
                    pxm = pmm.tile([96, CH], F32, tag="mm", space="PSUM")
                    phm = pmm.tile([96, CH], F32, tag="mm", space="PSUM")
                    for k in range(CH // NMM):
                        nc.tensor.matmul(pxm[:, k * NMM:(k + 1) * NMM],
                                         w_ap, agT[:, k * NMM:(k + 1) * NMM])
                    for k in range(CH // NMM):
                        nc.tensor.matmul(phm[:, k * NMM:(k + 1) * NMM],
                                         u_ap, lsc[:, k * NMM:(k + 1) * NMM])
                    # TT requires equal base partitions on both SB inputs;
                    # the schedule below realigns operands via ACT placement.
                    xm = io.tile([96, CH], F32, tag="xm")
                    hm = io.tile([96, CH], F32, tag="hm")
                    sc = ab.tile([32, CH], F32, tag="sc")
                    nc.scalar.activation(xm[:], pxm[:], AF.Identity,
                                         bias=b0_ap)
                    nc.scalar.activation(hm[:], phm[:], AF.Identity,
                                         bias=b1_ap)
                    # t1: xm[0:64] += hm[0:64]  (z and r pre-activations)
                    nc.vector.tensor_tensor(xm[0:64, :], xm[0:64, :],
                                            hm[0:64, :], ALU.add)
                    # z@xm[0:32], r@xm[32:64]
                    nc.scalar.activation(xm[0:64, :], xm[0:64, :], AF.Sigmoid)
                    # realign hh to base 32 (hm[0:64] is dead now)
                    nc.scalar.activation(hm[32:64, :], hm[64:96, :],
                                         AF.Identity)
                    # rh = r*hh -> hm[64:96] (base-64 for the t2 add)
                    nc.vector.tensor_tensor(hm[64:96, :], xm[32:64, :],
                                            hm[32:64, :], ALU.mult)
                    # t2 = xh + rh -> xm[64:96]
                    nc.vector.tensor_tensor(xm[64:96, :], xm[64:96, :],
                                            hm[64:96, :], ALU.add)
                    # cand -> hm[0:32] (base 0)
                    nc.scalar.activation(hm[0:32, :], xm[64:96, :], AF.Tanh)
                    # dd = ls - cand -> sc;  e2 = z*dd -> sc
                    nc.vector.tensor_tensor(sc[:], lsc[:], hm[0:32, :],
                                            ALU.subtract)
                    nc.vector.tensor_tensor(sc[:], xm[0:32, :], sc[:],
                                            ALU.mult)
                    # ls' = cand + e2 -> lsc
                    nc.vector.tensor_tensor(lsc[:], hm[0:32, :], sc[:],
                                            ALU.add)
                    nc.sync.dma_start(dst[:, o:o + CH], lsc[:])

    nc.compile()
    return nc


# ---------------------------------------------------------------------------
# Host-side glue
# ---------------------------------------------------------------------------

def _selu_np(x):
    return (LAM * (np.maximum(x, 0.0)
            + ALPHA * (np.expm1(np.minimum(x, 0.0))))).astype(np.float32)


def make_in_maps(cfg, pre, states_action, Wm, bm, W, U, b):
    Wm = np.asarray(Wm, np.float32)
    wpack = np.zeros((96, 260), np.float32)
    wpack[0:32, 0:32] = Wm[:32]
    wpack[0:32, 32:64] = Wm[32:]
    wpack[0:32, 64] = np.asarray(bm, np.float32)
    wpack[0:32, 65:161] = np.asarray(W, np.float32)
    wpack[0:32, 161:257] = np.asarray(U, np.float32)
    wpack[0:96, 257] = np.asarray(b[0], np.float32)
    wpack[0:96, 258] = np.asarray(b[1], np.float32)
    in_maps = []
    for c in range(cfg.NC):
        order = pre["orders"][c]
        ls0 = np.ascontiguousarray(
            np.asarray(states_action, np.float32)[c * cfg.EL + order].T)
        in_maps.append({
            "ls0": ls0,
            "idx": pre["idx"][c],
            "corr": pre["corr"][c],
            "wpack": wpack,
        })
    return in_maps


def readout_host(cfg, pre, ls_outs, states_graph_ids, num_graphs,
                 Wr1, br1, Wr2, br2, Wr3, br3):
    E, EL = cfg.E, cfg.EL
    ls_full = np.empty((E, 32), np.float32)
    for c in range(cfg.NC):
        order = pre["orders"][c]
        ls_full[c * EL + order] = ls_outs[c].T
    gids = np.asarray(states_graph_ids, np.int64)
    gs = np.zeros((num_graphs, 32), np.float32)
    np.add.at(gs, gids, ls_full)
    h = _selu_np(gs @ np.asarray(Wr1, np.float32) + np.asarray(br1, np.float32))
    h = _selu_np(h @ np.asarray(Wr2, np.float32) + np.asarray(br2, np.float32))
    a = np.maximum(h @ np.asarray(Wr3, np.float32)
                   + np.asarray(br3, np.float32), 0.0)
    return a.astype(np.float32)


_BUILD_CACHE = {}


def run_device(cfg, pre, in_maps, use_sim=False, trace=False):
    key = (cfg.E, cfg.T, tuple(pre["C_r"]))
    if key not in _BUILD_CACHE:
        _BUILD_CACHE[key] = build_full(cfg, pre["C_r"], pre["CTOT"])
    nc = _BUILD_CACHE[key]
    if use_sim:
        from concourse.bass_interp import MultiCoreSim
        sim = MultiCoreSim(nc, num_cores=cfg.NC)
        for c in range(cfg.NC):
            for k, v in in_maps[c].items():
                sim.cores[c].tensor(k)[:] = v
        sim.simulate()
        outs = [np.array(sim.cores[c].mem_tensor("ls_out"))
                for c in range(cfg.NC)]
        return outs, None
    res = run_bass_kernel_spmd(nc, in_maps, core_ids=list(range(cfg.NC)),
                               trace=trace)
    outs = [res.results[c]["ls_out"] for c in range(cfg.NC)]
    return outs, res


def _kernel_impl(inputs, use_sim=False, T=8, num_graphs=64, trace=False):
    states_action = np.asarray(inputs["states_action"], np.float32)
    E = states_action.shape[0]
    cfg = Cfg(E=E, T=T, n_cores=8)
    pre = preprocess(cfg, inputs["states_first"], inputs["states_second"])
    in_maps = make_in_maps(cfg, pre, states_action, inputs["Wm"],
                           inputs["bm"], inputs["W"], inputs["U"], inputs["b"])
    ls_outs, res = run_device(cfg, pre, in_maps, use_sim=use_sim, trace=trace)
    a = readout_host(cfg, pre, ls_outs, inputs["states_graph_ids"], num_graphs,
                     inputs["Wr1"], inputs["br1"], inputs["Wr2"],
                     inputs["br2"], inputs["Wr3"], inputs["br3"])
    return a, res


def kernel(**inputs):
    a, _ = _kernel_impl(inputs)
    return a



# revision 8
# speedup vs baseline: 1.1082x; 1.1082x over previous
"""Trainium2 Bass kernel for GNN message passing (nn_Actor_71141838291282).

Algorithm (per message-passing iteration, T=8):
    msg  = selu(ls[first] @ Wm1 + ls[second] @ Wm2 + bm)     [M, 32]
    agg  = segment_sum(msg, second, E)                        [E, 32]
    ls   = GRU(agg, ls)                                       [E, 32]
Readout: graph segment-sum + 3-layer MLP (done host-side; negligible work).

Distribution: 8 NeuronCores, shard by DESTINATION node (states_second).
Core c owns dests [c*EL, (c+1)*EL), EL = E/8 = 32768.
Per iteration each core:
  1. A = ls_loc @ Wm1 + bm (fp16, node-major) -> DRAM;  B = ls_loc @ Wm2 (fp16, SBUF)
  2. AllGather A -> full table A_ext [E, 32] fp16 in local DRAM
  3. "Rounds": dests per core are relabeled by descending in-degree (host-side
     permutation), so round r = the r-th edge of every dest with degree > r is a
     contiguous PREFIX of the dest space.  For each round: prefill a buffer with
     B[dest], indirect-DMA gather-add A_ext[first] (cce add) into it, apply SELU
     (exp(min(x,0)) composite, no selects), accumulate into agg (f32).
     Pad slots gather a -3e4 pad row => selu == -lam*alpha exactly; corrected by
     initializing agg with +lam*alpha*npad (static).
  4. GRU feat-major via PE matmuls (W/U stationary) + DVE/ACT elementwise.

All indices/permutations are computed host-side in numpy (static data).
"""

import math
import numpy as np

import concourse.bass as bass
import concourse.mybir as mybir
import concourse.tile as tile
from concourse import bacc
from concourse.bass_utils import run_bass_kernel_spmd

F32 = mybir.dt.float32
F16 = mybir.dt.float16
I32 = mybir.dt.int32

LAM = 1.0507009873554805
ALPHA = 1.6732632423543772
PAD_VAL = -30000.0  # pad row value in A table (f16 range)

P = 128
# NOTE: the HW DGE reads ONE index per partition per indirect DMA and
# fetches contiguous rows for multi-column offset APs, so batching
# columns into one instruction is NOT possible (verified by probe).
GATHER_BATCH = 1  # offset-AP columns per indirect DMA instruction


class Cfg:
    def __init__(self, E=262144, T=8, n_cores=8):
        self.E = E
        self.T = T
        self.NC = n_cores
        self.EL = E // n_cores
        self.C = self.EL // P            # grid cols (dest rank i at (i%P, i//P))
        assert self.EL % P == 0


# ---------------------------------------------------------------------------
# Host preprocessing
# ---------------------------------------------------------------------------

def preprocess(cfg, states_first, states_second):
    """Build per-core static index data.

    Returns dict with:
      order[c]   : [EL]   local node id for dest rank i  (descending degree)
      gperm      : [E]    global A-table row for global node u
      idx[c]     : [P, CTOT] int32 gather rows per round (concatenated cols)
      corr[c]    : [P, C] f32  = +LAM*ALPHA * npad  (agg init)
      C_r        : list of per-round col counts (compile-time consts)
    """
    E, EL, NC, C = cfg.E, cfg.EL, cfg.NC, cfg.C
    first = np.asarray(states_first, dtype=np.int64)
    second = np.asarray(states_second, dtype=np.int64)

    core_of = second // EL
    orders = []
    ranks = np.empty(E, dtype=np.int64)
    degs_sorted = []
    per_core_edges = []
    for c in range(NC):
        m = core_of == c
        ef = first[m]
        es = second[m] - c * EL
        deg = np.bincount(es, minlength=EL)
        order = np.argsort(-deg, kind="stable")        # rank -> local id
        rank = np.empty(EL, dtype=np.int64)
        rank[order] = np.arange(EL)
        orders.append(order)
        ranks[c * EL:(c + 1) * EL] = rank
        degs_sorted.append(deg[order])                  # descending
        per_core_edges.append((ef, es, rank))

    # global A-table row of node u (per-core perm layout, rank-ordered chunks)
    gperm = (np.arange(E) // EL) * EL + ranks

    maxdeg = max(int(d[0]) for d in degs_sorted)
    R = maxdeg
    # per-round edge counts n_r per core; global padded col counts
    C_r = []
    for r in range(R):
        n_r_max = max(int(np.count_nonzero(d > r)) for d in degs_sorted)
        C_r.append(max(1, math.ceil(n_r_max / P)))
    CTOT = sum(C_r)

    idx_all = []
    corr_all = []
    for c in range(NC):
        ef, es, rank = per_core_edges[c]
        d_sorted = degs_sorted[c]
        # sort edges by (dest rank, arbitrary); round index = occurrence count
        dest_rank = rank[es]
        o = np.argsort(dest_rank, kind="stable")
        dr = dest_rank[o]
        rows = gperm[ef[o]]
        # occurrence number within each dest
        occ = np.arange(len(dr)) - np.concatenate(
            ([0], np.cumsum(np.bincount(dr, minlength=EL))))[dr]
        idx = np.full((R, EL), cfg.E, dtype=np.int64)   # pad row = E
        idx[occ, dr] = rows
        # build [P, CTOT] layout: round r slots i in [0, P*C_r[r]), slot i=(p+P*cc)
        cols = np.zeros((P, CTOT), dtype=np.int32)
        off = 0
        npad = np.zeros(EL, dtype=np.int64)
        for r in range(R):
            ncols = C_r[r]
            sl = idx[r, :P * ncols]                       # slot i -> row
            pads = sl == cfg.E
            npad[:P * ncols] += pads
            cols[:, off:off + ncols] = sl.reshape(ncols, P).T
            off += ncols
        idx_all.append(cols)
        corr = (LAM * ALPHA) * npad.astype(np.float32)
        corr_all.append(corr.reshape(C, P).T.copy())     # [P, C]

    return dict(orders=orders, gperm=gperm, idx=idx_all, corr=corr_all,
                C_r=C_r, R=R, CTOT=CTOT)


# ---------------------------------------------------------------------------
# Device kernel builder
# ---------------------------------------------------------------------------

def build_full(cfg, C_r, CTOT):
    """Build the complete SPMD graph (all 8 cores run this identically).

    Column-block-major schedule: phase 3 (gather rounds) runs per block of
    W grid columns; as soon as a block's agg is final, its GRU chunks and
    the NEXT iteration's A/B matmuls for those nodes run — pipelined under
    the (GpSimd-bound) gathers of the remaining blocks.  The AllGather is
    the only global barrier per iteration.
    """
    E, EL, NC, C, T = cfg.E, cfg.EL, cfg.NC, cfg.C, cfg.T
    R = len(C_r)
    nc = bacc.Bacc("TRN2", target_bir_lowering=False, debug=False,
                   num_devices=NC)

    ls0 = nc.dram_tensor("ls0", [32, EL], F32, kind="ExternalInput")
    idx_in = nc.dram_tensor("idx", [P, CTOT], I32, kind="ExternalInput")
    corr_in = nc.dram_tensor("corr", [P, C], F32, kind="ExternalInput")
    # packed weights: wm1[0:32,0:32] wm2[0:32,32:64] bm[0:32,64] w[0:32,65:161]
    # u[0:32,161:257] b0[0:96,257] b1[0:96,258]
    wp_in = nc.dram_tensor("wpack", [96, 260], F32, kind="ExternalInput")
    ls_out = nc.dram_tensor("ls_out", [32, EL], F32, kind="ExternalOutput")

    a_loc = nc.dram_tensor("a_loc", [EL, 32], F16)
    a_ext = nc.dram_tensor("a_ext", [E + P, 32], F16, addr_space="Shared")
    ls_ping = nc.dram_tensor("ls_ping", [32, EL], F32)
    ls_pong = nc.dram_tensor("ls_pong", [32, EL], F32)

    CH = min(2048, EL)       # chunk (free dim) for A/B and GRU phases
    NMM = min(512, CH)       # matmul free dim
    n_ch = EL // CH
    GCOL = CH // P           # grid cols per chunk
    W = min(32, C)           # grid cols per phase-3 block
    NB = C // W              # number of blocks
    CPB = W // GCOL          # GRU chunks per block
    assert C % W == 0 and W % GCOL == 0

    # per-round column offsets into the idx layout
    off_r = [0]
    for r in range(R):
        off_r.append(off_r[-1] + C_r[r])

    AF = mybir.ActivationFunctionType
    ALU = mybir.AluOpType

    with tile.TileContext(nc) as tc:
        with (
            tc.tile_pool(name="sb", bufs=1) as sb,
            tc.tile_pool(name="io", bufs=2) as io,
            tc.tile_pool(name="ab", bufs=2) as ab,
            tc.tile_pool(name="rp", bufs=2) as rp,
            tc.tile_pool(name="ag", bufs=2) as agp,
            tc.tile_pool(name="mm", bufs=2, space="PSUM") as pmm,
        ):
            # ---- persistent SBUF ----
            idx_sb = sb.tile([P, CTOT], I32, tag="idx")
            nc.sync.dma_start(idx_sb[:], idx_in[:, :])
            corr_sb = sb.tile([P, C], F32, tag="corr")
            nc.sync.dma_start(corr_sb[:], corr_in[:, :])
            wp = sb.tile([96, 260], F32, tag="wp")
            nc.sync.dma_start(wp[:], wp_in[:, :])
            wm1 = wp[0:32, 0:32]
            wm2 = wp[0:32, 32:64]
            bm_ap = wp[0:32, 64:65]
            w_ap = wp[0:32, 65:161]
            u_ap = wp[0:32, 161:257]
            b0_ap = wp[0:96, 257:258]
            b1_ap = wp[0:96, 258:259]

            b_nm = sb.tile([P, C, 32], F16, tag="b_nm")

            padrow = sb.tile([P, 32], F16, tag="padrow")
            nc.vector.memset(padrow[:], PAD_VAL)
            nc.sync.dma_start(a_ext[E:E + P, :], padrow[:])

            a_loc3 = a_loc[:, :].rearrange("(c p) f -> p c f", p=P)

            def ls_src(t):
                if t == 0:
                    return ls0
                return ls_ping if t % 2 == 1 else ls_pong

            def ls_dst(t):
                if t == cfg.T - 1:
                    return ls_out
                return ls_ping if t % 2 == 0 else ls_pong

            def phase1_chunk(lsc, ch):
                """A/B for node chunk ch from feat-major ls tile lsc."""
                pa = pmm.tile([32, CH], F32, tag="mm", space="PSUM")
                pb = pmm.tile([32, CH], F32, tag="mm", space="PSUM")
                for k in range(CH // NMM):
                    nc.tensor.matmul(pa[:, k * NMM:(k + 1) * NMM],
                                     wm1, lsc[:, k * NMM:(k + 1) * NMM])
                for k in range(CH // NMM):
                    nc.tensor.matmul(pb[:, k * NMM:(k + 1) * NMM],
                                     wm2, lsc[:, k * NMM:(k + 1) * NMM])
                aT = ab.tile([32, CH], F16, tag="aT")
                bT = ab.tile([32, CH], F16, tag="bT")
                nc.scalar.activation(aT[:], pa[:], AF.Identity, bias=bm_ap)
                nc.scalar.activation(bT[:], pb[:], AF.Identity)
                # 32x32 block transpose + block remap into node-major grid
                # tmp[q, 32k+f] = srcT[f, 32k+q];  node = ch*CH + 32k + q;
                # grid (p, c) = (32*(k%4)+q, ch*GCOL + k//4)
                a_stg = ab.tile([P, GCOL, 32], F16, tag="a_stg")
                for srcT, dstG, c0 in ((aT, a_stg, 0),
                                       (bT, b_nm, ch * GCOL)):
                    tT = ab.tile([32, CH], F16, tag="tT")
                    nc.vector.transpose(tT[:], srcT[:])
                    t3 = tT[:].rearrange("q (k f) -> q k f", f=32)
                    for p32 in range(4):
                        nc.vector.tensor_copy(
                            dstG[32 * p32:32 * (p32 + 1),
                                 c0:c0 + GCOL, :],
                            t3[:, p32::4, :])
                # A chunk -> DRAM table rows (row i = p + P*c)
                nc.sync.dma_start(
                    a_loc3[:, ch * GCOL:(ch + 1) * GCOL, :], a_stg[:])

            def phase4_chunk(agg_ap, ch, src, dst):
                """GRU for node chunk ch; returns feat-major ls' tile."""
                o = ch * CH
                agT = io.tile([32, CH], F32, tag="agT")
                tmp2 = io.tile([P, GCOL * 32], F32, tag="tmp2")
                nc.vector.transpose(
                    tmp2[:], agg_ap.rearrange("p a b -> p (a b)"))
                tmp23 = tmp2[:].rearrange("p (c q) -> p c q", q=32)
                agT3 = agT[:].rearrange("f (c w) -> f c w", w=P)
                for p32 in range(4):
                    nc.vector.tensor_copy(
                        agT3[:, :, 32 * p32:32 * (p32 + 1)],
                        tmp23[32 * p32:32 * (p32 + 1), :, :])
                lsc = io.tile([32, CH], F32, tag="lsio")
                nc.sync.dma_start(lsc[:], src[:, o:o + CH])
                pxm = pmm.tile([96, CH], F32, tag="mm", space="PSUM")
                phm = pmm.tile([96, CH], F32, tag="mm", space="PSUM")
                for k in range(CH // NMM):
                    nc.tensor.matmul(pxm[:, k * NMM:(k + 1) * NMM],
                                     w_ap, agT[:, k * NMM:(k + 1) * NMM])
                for k in range(CH // NMM):
                    nc.tensor.matmul(phm[:, k * NMM:(k + 1) * NMM],
                                     u_ap, lsc[:, k * NMM:(k + 1) * NMM])
                # TT requires equal base partitions on both SB inputs;
                # the schedule below realigns operands via ACT placement.
                xm = io.tile([96, CH], F32, tag="xm")
                hm = io.tile([96, CH], F32, tag="hm")
                sc = ab.tile([32, CH], F32, tag="sc")
                nc.scalar.activation(xm[:], pxm[:], AF.Identity, bias=b0_ap)
                nc.scalar.activation(hm[:], phm[:], AF.Identity, bias=b1_ap)
                # t1: xm[0:64] += hm[0:64]  (z and r pre-activations)
                nc.vector.tensor_tensor(xm[0:64, :], xm[0:64, :],
                                        hm[0:64, :], ALU.add)
                # z@xm[0:32], r@xm[32:64]
                nc.scalar.activation(xm[0:64, :], xm[0:64, :], AF.Sigmoid)
                # realign hh to base 32 (hm[0:64] is dead now)
                nc.scalar.activation(hm[32:64, :], hm[64:96, :], AF.Identity)
                # rh = r*hh -> hm[64:96] (base-64 for the t2 add)
                nc.vector.tensor_tensor(hm[64:96, :], xm[32:64, :],
                                        hm[32:64, :], ALU.mult)
                # t2 = xh + rh -> xm[64:96]
                nc.vector.tensor_tensor(xm[64:96, :], xm[64:96, :],
                                        hm[64:96, :], ALU.add)
                # cand -> hm[0:32] (base 0)
                nc.scalar.activation(hm[0:32, :], xm[64:96, :], AF.Tanh)
                # dd = ls - cand -> sc;  e2 = z*dd -> sc
                nc.vector.tensor_tensor(sc[:], lsc[:], hm[0:32, :],
                                        ALU.subtract)
                nc.vector.tensor_tensor(sc[:], xm[0:32, :], sc[:], ALU.mult)
                # ls' = cand + e2 -> lsc
                nc.vector.tensor_tensor(lsc[:], hm[0:32, :], sc[:], ALU.add)
                nc.sync.dma_start(dst[:, o:o + CH], lsc[:])
                return lsc

            # ---- t=0 A/B from ls0 ----
            for ch in range(n_ch):
                lsc = io.tile([32, CH], F32, tag="lsio")
                nc.sync.dma_start(lsc[:], ls0[:, ch * CH:(ch + 1) * CH])
                phase1_chunk(lsc, ch)

            for t in range(cfg.T):
                src = ls_src(t)
                dst = ls_dst(t)

                # ---- AllGather: a_loc -> full table a_ext ----
                nc.gpsimd.collective_compute(
                    "AllGather", ALU.bypass,
                    replica_groups=[list(range(NC))],
                    ins=[a_loc[:, :].opt()],
                    outs=[a_ext[0:E, :].opt()],
                )

                for cb in range(NB):
                    cb0 = cb * W
                    # ---- phase 3: gather rounds for this column block ----
                    agg = agp.tile([P, W, 32], F32, tag="agg")
                    nc.vector.tensor_copy(
                        agg[:], corr_sb[:, cb0:cb0 + W, None]
                        .to_broadcast([P, W, 32]))
                    for r in range(R):
                        c_hi = min(C_r[r], cb0 + W)
                        if c_hi <= cb0:
                            break          # C_r is non-increasing
                        cw = c_hi - cb0
                        rb = rp.tile([P, W, 32], F16, tag="rb")
                        rbv = rb[:, 0:cw, :]
                        rbf = rb[:].rearrange("p a b -> p (a b)")
                        nc.vector.tensor_copy(rbv, b_nm[:, cb0:c_hi, :])
                        # indirect gather: one column per instruction (the
                        # HW DGE supports only one offset per partition)
                        for cc in range(cw):
                            col = off_r[r] + cb0 + cc
                            nc.gpsimd.indirect_dma_start(
                                out=rbf[:, cc * 32:(cc + 1) * 32],
                                out_offset=None,
                                in_=a_ext[:, :],
                                in_offset=bass.IndirectOffsetOnAxis(
                                    ap=idx_sb[:, col:col + 1], axis=0),
                                compute_op=ALU.add,
                            )
                        # selu: m=min(x,0); v=LAM*max(x,0) (in-place rb);
                        # e=exp(m); s=LAM*ALPHA*e-LAM*ALPHA; s+=v; agg+=s
                        mt = rp.tile([P, W, 32], F16, tag="mt")
                        mtv = mt[:, 0:cw, :]
                        nc.vector.tensor_scalar(mtv, rbv, 0.0, None, ALU.min)
                        nc.vector.tensor_scalar(rbv, rbv, 0.0, LAM, ALU.max,
                                                ALU.mult)
                        nc.scalar.activation(mtv, mtv, AF.Exp)
                        nc.vector.tensor_scalar(mtv, mtv, LAM * ALPHA,
                                                -LAM * ALPHA, ALU.mult,
                                                ALU.add)
                        nc.vector.tensor_tensor(mtv, mtv, rbv, ALU.add)
                        av = agg[:, 0:cw, :]
                        nc.vector.tensor_tensor(av, av, mtv, ALU.add)

                    # ---- phase 4 (+ next iteration's A/B) per chunk ----
                    for j in range(CPB):
                        ch = cb * CPB + j
                        lsc = phase4_chunk(
                            agg[:, j * GCOL:(j + 1) * GCOL, :], ch, src, dst)
                        if t < cfg.T - 1:
                            phase1_chunk(lsc, ch)

    nc.compile()
    return nc


# ---------------------------------------------------------------------------
# Host-side glue
# ---------------------------------------------------------------------------

def _selu_np(x):
    return (LAM * (np.maximum(x, 0.0)
            + ALPHA * (np.expm1(np.minimum(x, 0.0))))).astype(np.float32)


def make_in_maps(cfg, pre, states_action, Wm, bm, W, U, b):
    Wm = np.asarray(Wm, np.float32)
    wpack = np.zeros((96, 260), np.float32)
    wpack[0:32, 0:32] = Wm[:32]
    wpack[0:32, 32:64] = Wm[32:]
    wpack[0:32, 64] = np.asarray(bm, np.float32)
    wpack[0:32, 65:161] = np.asarray(W, np.float32)
    wpack[0:32, 161:257] = np.asarray(U, np.float32)
    wpack[0:96, 257] = np.asarray(b[0], np.float32)
    wpack[0:96, 258] = np.asarray(b[1], np.float32)
    in_maps = []
    for c in range(cfg.NC):
        order = pre["orders"][c]
        ls0 = np.ascontiguousarray(
            np.asarray(states_action, np.float32)[c * cfg.EL + order].T)
        in_maps.append({
            "ls0": ls0,
            "idx": pre["idx"][c],
            "corr": pre["corr"][c],
            "wpack": wpack,
        })
    return in_maps


def readout_host(cfg, pre, ls_outs, states_graph_ids, num_graphs,
                 Wr1, br1, Wr2, br2, Wr3, br3):
    E, EL = cfg.E, cfg.EL
    ls_full = np.empty((E, 32), np.float32)
    for c in range(cfg.NC):
        order = pre["orders"][c]
        ls_full[c * EL + order] = ls_outs[c].T
    gids = np.asarray(states_graph_ids, np.int64)
    gs = np.zeros((num_graphs, 32), np.float32)
    np.add.at(gs, gids, ls_full)
    h = _selu_np(gs @ np.asarray(Wr1, np.float32) + np.asarray(br1, np.float32))
    h = _selu_np(h @ np.asarray(Wr2, np.float32) + np.asarray(br2, np.float32))
    a = np.maximum(h @ np.asarray(Wr3, np.float32)
                   + np.asarray(br3, np.float32), 0.0)
    return a.astype(np.float32)


_BUILD_CACHE = {}


def run_device(cfg, pre, in_maps, use_sim=False, trace=False):
    key = (cfg.E, cfg.T, tuple(pre["C_r"]))
    if key not in _BUILD_CACHE:
        _BUILD_CACHE[key] = build_full(cfg, pre["C_r"], pre["CTOT"])
    nc = _BUILD_CACHE[key]
    if use_sim:
        from concourse.bass_interp import MultiCoreSim
        sim = MultiCoreSim(nc, num_cores=cfg.NC)
        for c in range(cfg.NC):
            for k, v in in_maps[c].items():
                sim.cores[c].tensor(k)[:] = v
        sim.simulate()
        outs = [np.array(sim.cores[c].mem_tensor("ls_out"))
                for c in range(cfg.NC)]
        return outs, None
    res = run_bass_kernel_spmd(nc, in_maps, core_ids=list(range(cfg.NC)),
                               trace=trace)
    outs = [res.results[c]["ls_out"] for c in range(cfg.NC)]
    return outs, res


def _kernel_impl(inputs, use_sim=False, T=8, num_graphs=64, trace=False):
    states_action = np.asarray(inputs["states_action"], np.float32)
    E = states_action.shape[0]
    cfg = Cfg(E=E, T=T, n_cores=8)
    pre = preprocess(cfg, inputs["states_first"], inputs["states_second"])
    in_maps = make_in_maps(cfg, pre, states_action, inputs["Wm"],
                           inputs["bm"], inputs["W"], inputs["U"], inputs["b"])
    ls_outs, res = run_device(cfg, pre, in_maps, use_sim=use_sim, trace=trace)
    a = readout_host(cfg, pre, ls_outs, inputs["states_graph_ids"], num_graphs,
                     inputs["Wr1"], inputs["br1"], inputs["Wr2"],
                     inputs["br2"], inputs["Wr3"], inputs["br3"])
    return a, res


def kernel(**inputs):
    a, _ = _kernel_impl(inputs)
    return a



# revision 13
# speedup vs baseline: 1.1090x; 1.0007x over previous
"""Trainium2 Bass kernel for GNN message passing (nn_Actor_71141838291282).

Algorithm (per message-passing iteration, T=8):
    msg  = selu(ls[first] @ Wm1 + ls[second] @ Wm2 + bm)     [M, 32]
    agg  = segment_sum(msg, second, E)                        [E, 32]
    ls   = GRU(agg, ls)                                       [E, 32]
Readout: graph segment-sum + 3-layer MLP (done host-side; negligible work).

Distribution: 8 NeuronCores, shard by DESTINATION node (states_second).
Core c owns dests [c*EL, (c+1)*EL), EL = E/8 = 32768.
Per iteration each core:
  1. A = ls_loc @ Wm1 + bm (fp16, node-major) -> DRAM;  B = ls_loc @ Wm2 (fp16, SBUF)
  2. AllGather A -> full table A_ext [E, 32] fp16 in local DRAM
  3. "Rounds": dests per core are relabeled by descending in-degree (host-side
     permutation), so round r = the r-th edge of every dest with degree > r is a
     contiguous PREFIX of the dest space.  For each round: prefill a buffer with
     B[dest], indirect-DMA gather-add A_ext[first] (cce add) into it, apply SELU
     (exp(min(x,0)) composite, no selects), accumulate into agg (f32).
     Pad slots gather a -3e4 pad row => selu == -lam*alpha exactly; corrected by
     initializing agg with +lam*alpha*npad (static).
  4. GRU feat-major via PE matmuls (W/U stationary) + DVE/ACT elementwise.

All indices/permutations are computed host-side in numpy (static data).
"""

import math
import numpy as np

import concourse.bass as bass
import concourse.mybir as mybir
import concourse.tile as tile
from concourse import bacc
from concourse.bass_utils import run_bass_kernel_spmd

F32 = mybir.dt.float32
F16 = mybir.dt.float16
I32 = mybir.dt.int32

LAM = 1.0507009873554805
ALPHA = 1.6732632423543772
PAD_VAL = -30000.0  # pad row value in A table (f16 range)

P = 128
# NOTE: the HW DGE reads ONE index per partition per indirect DMA and
# fetches contiguous rows for multi-column offset APs, so batching
# columns into one instruction is NOT possible (verified by probe).
GATHER_BATCH = 1  # offset-AP columns per indirect DMA instruction


class Cfg:
    def __init__(self, E=262144, T=8, n_cores=8):
        self.E = E
        self.T = T
        self.NC = n_cores
        self.EL = E // n_cores
        self.C = self.EL // P            # grid cols (dest rank i at (i%P, i//P))
        assert self.EL % P == 0


# ---------------------------------------------------------------------------
# Host preprocessing
# ---------------------------------------------------------------------------

def preprocess(cfg, states_first, states_second):
    """Build per-core static index data.

    Returns dict with:
      order[c]   : [EL]   local node id for dest rank i  (descending degree)
      gperm      : [E]    global A-table row for global node u
      idx[c]     : [P, CTOT] int32 gather rows per round (concatenated cols)
      corr[c]    : [P, C] f32  = +LAM*ALPHA * npad  (agg init)
      C_r        : list of per-round col counts (compile-time consts)
    """
    E, EL, NC, C = cfg.E, cfg.EL, cfg.NC, cfg.C
    first = np.asarray(states_first, dtype=np.int64)
    second = np.asarray(states_second, dtype=np.int64)

    core_of = second // EL
    orders = []
    ranks = np.empty(E, dtype=np.int64)
    degs_sorted = []
    per_core_edges = []
    for c in range(NC):
        m = core_of == c
        ef = first[m]
        es = second[m] - c * EL
        deg = np.bincount(es, minlength=EL)
        order = np.argsort(-deg, kind="stable")        # rank -> local id
        rank = np.empty(EL, dtype=np.int64)
        rank[order] = np.arange(EL)
        orders.append(order)
        ranks[c * EL:(c + 1) * EL] = rank
        degs_sorted.append(deg[order])                  # descending
        per_core_edges.append((ef, es, rank))

    # global A-table row of node u.  Layout: [half][core][rank % (EL/2)] so
    # each half of the table is AllGather-able independently (half h of the
    # table = concat over cores of their local ranks [h*EL/2, (h+1)*EL/2)).
    H = EL // 2
    cores = np.arange(E) // EL
    halves = ranks // H
    gperm = halves * (E // 2) + cores * H + (ranks % H)

    maxdeg = max(int(d[0]) for d in degs_sorted)
    R = maxdeg
    # per-round edge counts n_r per core; global padded col counts
    C_r = []
    for r in range(R):
        n_r_max = max(int(np.count_nonzero(d > r)) for d in degs_sorted)
        C_r.append(max(1, math.ceil(n_r_max / P)))
    CTOT = sum(C_r)

    idx_all = []
    corr_all = []
    for c in range(NC):
        ef, es, rank = per_core_edges[c]
        d_sorted = degs_sorted[c]
        # sort edges by (dest rank, arbitrary); round index = occurrence count
        dest_rank = rank[es]
        o = np.argsort(dest_rank, kind="stable")
        dr = dest_rank[o]
        rows = gperm[ef[o]]
        # occurrence number within each dest
        occ = np.arange(len(dr)) - np.concatenate(
            ([0], np.cumsum(np.bincount(dr, minlength=EL))))[dr]
        idx = np.full((R, EL), cfg.E, dtype=np.int64)   # pad row = E
        idx[occ, dr] = rows
        # build [P, CTOT] layout: round r slots i in [0, P*C_r[r]), slot i=(p+P*cc)
        cols = np.zeros((P, CTOT), dtype=np.int32)
        off = 0
        npad = np.zeros(EL, dtype=np.int64)
        for r in range(R):
            ncols = C_r[r]
            sl = idx[r, :P * ncols]                       # slot i -> row
            pads = sl == cfg.E
            npad[:P * ncols] += pads
            cols[:, off:off + ncols] = sl.reshape(ncols, P).T
            off += ncols
        idx_all.append(cols)
        corr = (LAM * ALPHA) * npad.astype(np.float32)
        corr_all.append(corr.reshape(C, P).T.copy())     # [P, C]

    return dict(orders=orders, gperm=gperm, idx=idx_all, corr=corr_all,
                C_r=C_r, R=R, CTOT=CTOT)


# ---------------------------------------------------------------------------
# Device kernel builder
# ---------------------------------------------------------------------------

def build_full(cfg, C_r, CTOT):
    """Build the complete SPMD graph (all 8 cores run this identically).

    Column-block-major schedule: phase 3 (gather rounds) runs per block of
    W grid columns; as soon as a block's agg is final, its GRU chunks and
    the NEXT iteration's A/B matmuls for those nodes run — pipelined under
    the (GpSimd-bound) gathers of the remaining blocks.  The AllGather is
    the only global barrier per iteration.
    """
    E, EL, NC, C, T = cfg.E, cfg.EL, cfg.NC, cfg.C, cfg.T
    R = len(C_r)
    nc = bacc.Bacc("TRN2", target_bir_lowering=False, debug=False,
                   num_devices=NC)

    ls0 = nc.dram_tensor("ls0", [32, EL], F32, kind="ExternalInput")
    idx_in = nc.dram_tensor("idx", [P, CTOT], I32, kind="ExternalInput")
    corr_in = nc.dram_tensor("corr", [P, C], F32, kind="ExternalInput")
    # packed weights: wm1[0:32,0:32] wm2[0:32,32:64] bm[0:32,64] w[0:32,65:161]
    # u[0:32,161:257] b0[0:96,257] b1[0:96,258]
    wp_in = nc.dram_tensor("wpack", [96, 260], F32, kind="ExternalInput")
    ls_out = nc.dram_tensor("ls_out", [32, EL], F32, kind="ExternalOutput")

    a_loc = nc.dram_tensor("a_loc", [EL, 32], F16)
    # double-buffered table: AG for iteration t+1 overlaps t's gathers
    a_exts = [nc.dram_tensor(f"a_ext{i}", [E + P, 32], F16,
                             addr_space="Shared") for i in range(2)]
    ls_ping = nc.dram_tensor("ls_ping", [32, EL], F32)
    ls_pong = nc.dram_tensor("ls_pong", [32, EL], F32)

    CH = min(2048, EL)       # chunk (free dim) for A/B and GRU phases
    NMM = min(512, CH)       # matmul free dim
    n_ch = EL // CH
    GCOL = CH // P           # grid cols per chunk
    W = min(16, C)           # grid cols per phase-3 block
    NB = C // W              # number of blocks
    CPB = max(1, W // GCOL)  # GRU chunks per block
    assert C % W == 0 and W % GCOL == 0

    # per-round column offsets into the idx layout
    off_r = [0]
    for r in range(R):
        off_r.append(off_r[-1] + C_r[r])

    AF = mybir.ActivationFunctionType
    ALU = mybir.AluOpType

    with tile.TileContext(nc) as tc:
        with (
            tc.tile_pool(name="sb", bufs=1) as sb,
            tc.tile_pool(name="io", bufs=2) as io,
            tc.tile_pool(name="ab", bufs=2) as ab,
            tc.tile_pool(name="rp", bufs=2) as rp,
            tc.tile_pool(name="ag", bufs=2) as agp,
            tc.tile_pool(name="mm", bufs=2, space="PSUM") as pmm,
        ):
            # ---- persistent SBUF ----
            idx_sb = sb.tile([P, CTOT], I32, tag="idx")
            nc.sync.dma_start(idx_sb[:], idx_in[:, :])
            corr_sb = sb.tile([P, C], F32, tag="corr")
            nc.sync.dma_start(corr_sb[:], corr_in[:, :])
            wp = sb.tile([96, 260], F32, tag="wp")
            nc.sync.dma_start(wp[:], wp_in[:, :])
            wm1 = wp[0:32, 0:32]
            wm2 = wp[0:32, 32:64]
            bm_ap = wp[0:32, 64:65]
            w_ap = wp[0:32, 65:161]
            u_ap = wp[0:32, 161:257]
            b0_ap = wp[0:96, 257:258]
            b1_ap = wp[0:96, 258:259]

            b_nm = sb.tile([P, C, 32], F16, tag="b_nm")

            padrow = sb.tile([P, 32], F16, tag="padrow")
            nc.vector.memset(padrow[:], PAD_VAL)
            for ae in a_exts:
                nc.sync.dma_start(ae[E:E + P, :], padrow[:])

            a_loc3 = a_loc[:, :].rearrange("(c p) f -> p c f", p=P)
            HL = EL // 2
            mid_ch = max(1, n_ch // 2)

            def allgather_half(half, dst_ext):
                nc.gpsimd.collective_compute(
                    "AllGather", ALU.bypass,
                    replica_groups=[list(range(NC))],
                    ins=[a_loc[half * HL:(half + 1) * HL, :].opt()],
                    outs=[dst_ext[half * (E // 2):
                                  (half + 1) * (E // 2), :].opt()],
                )

            def ls_src(t):
                if t == 0:
                    return ls0
                return ls_ping if t % 2 == 1 else ls_pong

            def ls_dst(t):
                if t == cfg.T - 1:
                    return ls_out
                return ls_ping if t % 2 == 0 else ls_pong

            def phase1_chunk(lsc, ch):
                """A/B for node chunk ch from feat-major ls tile lsc."""
                pa = pmm.tile([32, CH], F32, tag="mm", space="PSUM")
                pb = pmm.tile([32, CH], F32, tag="mm", space="PSUM")
                for k in range(CH // NMM):
                    nc.tensor.matmul(pa[:, k * NMM:(k + 1) * NMM],
                                     wm1, lsc[:, k * NMM:(k + 1) * NMM])
                for k in range(CH // NMM):
                    nc.tensor.matmul(pb[:, k * NMM:(k + 1) * NMM],
                                     wm2, lsc[:, k * NMM:(k + 1) * NMM])
                aT = ab.tile([32, CH], F16, tag="aT")
                bT = ab.tile([32, CH], F16, tag="bT")
                nc.scalar.activation(aT[:], pa[:], AF.Identity, bias=bm_ap)
                nc.scalar.activation(bT[:], pb[:], AF.Identity)
                # 32x32 block transpose + block remap into node-major grid
                # tmp[q, 32k+f] = srcT[f, 32k+q];  node = ch*CH + 32k + q;
                # grid (p, c) = (32*(k%4)+q, ch*GCOL + k//4)
                a_stg = ab.tile([P, GCOL, 32], F16, tag="a_stg")
                for srcT, dstG, c0 in ((aT, a_stg, 0),
                                       (bT, b_nm, ch * GCOL)):
                    tT = ab.tile([32, CH], F16, tag="tT")
                    nc.vector.transpose(tT[:], srcT[:])
                    t3 = tT[:].rearrange("q (k f) -> q k f", f=32)
                    for p32 in range(4):
                        nc.vector.tensor_copy(
                            dstG[32 * p32:32 * (p32 + 1),
                                 c0:c0 + GCOL, :],
                            t3[:, p32::4, :])
                # A chunk -> DRAM table rows (row i = p + P*c)
                nc.sync.dma_start(
                    a_loc3[:, ch * GCOL:(ch + 1) * GCOL, :], a_stg[:])

            def phase4_chunk(agg_ap, ch, src, dst):
                """GRU for node chunk ch; returns feat-major ls' tile."""
                o = ch * CH
                agT = io.tile([32, CH], F32, tag="agT")
                tmp2 = io.tile([P, GCOL * 32], F32, tag="tmp2")
                nc.vector.transpose(
                    tmp2[:], agg_ap.rearrange("p a b -> p (a b)"))
                tmp23 = tmp2[:].rearrange("p (c q) -> p c q", q=32)
                agT3 = agT[:].rearrange("f (c w) -> f c w", w=P)
                for p32 in range(4):
                    nc.vector.tensor_copy(
                        agT3[:, :, 32 * p32:32 * (p32 + 1)],
                        tmp23[32 * p32:32 * (p32 + 1), :, :])
                lsc = io.tile([32, CH], F32, tag="lsio")
                nc.sync.dma_start(lsc[:], src[:, o:o + CH])
                pxm = pmm.tile([96, CH], F32, tag="mm", space="PSUM")
                phm = pmm.tile([96, CH], F32, tag="mm", space="PSUM")
                for k in range(CH // NMM):
                    nc.tensor.matmul(pxm[:, k * NMM:(k + 1) * NMM],
                                     w_ap, agT[:, k * NMM:(k + 1) * NMM])
                for k in range(CH // NMM):
                    nc.tensor.matmul(phm[:, k * NMM:(k + 1) * NMM],
                                     u_ap, lsc[:, k * NMM:(k + 1) * NMM])
                # TT requires equal base partitions on both SB inputs;
                # the schedule below realigns operands via ACT placement.
                xm = io.tile([96, CH], F32, tag="xm")
                hm = io.tile([96, CH], F32, tag="hm")
                sc = ab.tile([32, CH], F32, tag="sc")
                nc.scalar.activation(xm[:], pxm[:], AF.Identity, bias=b0_ap)
                nc.scalar.activation(hm[:], phm[:], AF.Identity, bias=b1_ap)
                # t1: xm[0:64] += hm[0:64]  (z and r pre-activations)
                nc.vector.tensor_tensor(xm[0:64, :], xm[0:64, :],
                                        hm[0:64, :], ALU.add)
                # z@xm[0:32], r@xm[32:64]
                nc.scalar.activation(xm[0:64, :], xm[0:64, :], AF.Sigmoid)
                # realign hh to base 32 (hm[0:64] is dead now)
                nc.scalar.activation(hm[32:64, :], hm[64:96, :], AF.Identity)
                # rh = r*hh -> hm[64:96] (base-64 for the t2 add)
                nc.vector.tensor_tensor(hm[64:96, :], xm[32:64, :],
                                        hm[32:64, :], ALU.mult)
                # t2 = xh + rh -> xm[64:96]
                nc.vector.tensor_tensor(xm[64:96, :], xm[64:96, :],
                                        hm[64:96, :], ALU.add)
                # cand -> hm[0:32] (base 0)
                nc.scalar.activation(hm[0:32, :], xm[64:96, :], AF.Tanh)
                # dd = ls - cand -> sc;  e2 = z*dd -> sc
                nc.vector.tensor_tensor(sc[:], lsc[:], hm[0:32, :],
                                        ALU.subtract)
                nc.vector.tensor_tensor(sc[:], xm[0:32, :], sc[:], ALU.mult)
                # ls' = cand + e2 -> lsc
                nc.vector.tensor_tensor(lsc[:], hm[0:32, :], sc[:], ALU.add)
                nc.sync.dma_start(dst[:, o:o + CH], lsc[:])
                return lsc

            # ---- t=0 A/B from ls0 (AG half 0 fires mid-way) ----
            for ch in range(n_ch):
                lsc = io.tile([32, CH], F32, tag="lsio")
                nc.sync.dma_start(lsc[:], ls0[:, ch * CH:(ch + 1) * CH])
                phase1_chunk(lsc, ch)
                if ch + 1 == mid_ch:
                    allgather_half(0, a_exts[0])
            allgather_half(1, a_exts[0])

            for t in range(cfg.T):
                src = ls_src(t)
                dst = ls_dst(t)
                a_ext = a_exts[t % 2]
                ae_next = a_exts[(t + 1) % 2]
                done_ch = 0

                for cb in range(NB):
                    cb0 = cb * W
                    # ---- phase 3: gather rounds for this column block ----
                    agg = agp.tile([P, W, 32], F32, tag="agg")
                    nc.vector.tensor_copy(
                        agg[:], corr_sb[:, cb0:cb0 + W, None]
                        .to_broadcast([P, W, 32]))
                    for r in range(R):
                        c_hi = min(C_r[r], cb0 + W)
                        if c_hi <= cb0:
                            break          # C_r is non-increasing
                        cw = c_hi - cb0
                        rb = rp.tile([P, W, 32], F16, tag="rb")
                        rbv = rb[:, 0:cw, :]
                        rbf = rb[:].rearrange("p a b -> p (a b)")
                        nc.vector.tensor_copy(rbv, b_nm[:, cb0:c_hi, :])
                        # indirect gather: one column per instruction (the
                        # HW DGE supports only one offset per partition)
                        for cc in range(cw):
                            col = off_r[r] + cb0 + cc
                            nc.gpsimd.indirect_dma_start(
                                out=rbf[:, cc * 32:(cc + 1) * 32],
                                out_offset=None,
                                in_=a_ext[:, :],
                                in_offset=bass.IndirectOffsetOnAxis(
                                    ap=idx_sb[:, col:col + 1], axis=0),
                                compute_op=ALU.add,
                            )
                        # selu: m=min(x,0); v=LAM*max(x,0) (in-place rb);
                        # e=exp(m); s=LAM*ALPHA*e-LAM*ALPHA; s+=v; agg+=s
                        mt = rp.tile([P, W, 32], F16, tag="mt")
                        mtv = mt[:, 0:cw, :]
                        nc.vector.tensor_scalar(mtv, rbv, 0.0, None, ALU.min)
                        nc.vector.tensor_scalar(rbv, rbv, 0.0, LAM, ALU.max,
                                                ALU.mult)
                        nc.scalar.activation(mtv, mtv, AF.Exp)
                        nc.vector.tensor_scalar(mtv, mtv, LAM * ALPHA,
                                                -LAM * ALPHA, ALU.mult,
                                                ALU.add)
                        nc.vector.tensor_tensor(mtv, mtv, rbv, ALU.add)
                        av = agg[:, 0:cw, :]
                        nc.vector.tensor_tensor(av, av, mtv, ALU.add)

                    # ---- phase 4 (+ next iteration's A/B) per chunk ----
                    for j in range(CPB):
                        ch = cb * CPB + j
                        lsc = phase4_chunk(
                            agg[:, j * GCOL:(j + 1) * GCOL, :], ch, src, dst)
                        if t < cfg.T - 1:
                            phase1_chunk(lsc, ch)
                            done_ch += 1
                            if done_ch == mid_ch:
                                allgather_half(0, ae_next)
                if t < cfg.T - 1:
                    allgather_half(1, ae_next)

    nc.compile()
    return nc


# ---------------------------------------------------------------------------
# Host-side glue
# ---------------------------------------------------------------------------

def _selu_np(x):
    return (LAM * (np.maximum(x, 0.0)
            + ALPHA * (np.expm1(np.minimum(x, 0.0))))).astype(np.float32)


def make_in_maps(cfg, pre, states_action, Wm, bm, W, U, b):
    Wm = np.asarray(Wm, np.float32)
    wpack = np.zeros((96, 260), np.float32)
    wpack[0:32, 0:32] = Wm[:32]
    wpack[0:32, 32:64] = Wm[32:]
    wpack[0:32, 64] = np.asarray(bm, np.float32)
    wpack[0:32, 65:161] = np.asarray(W, np.float32)
    wpack[0:32, 161:257] = np.asarray(U, np.float32)
    wpack[0:96, 257] = np.asarray(b[0], np.float32)
    wpack[0:96, 258] = np.asarray(b[1], np.float32)
    in_maps = []
    for c in range(cfg.NC):
        order = pre["orders"][c]
        ls0 = np.ascontiguousarray(
            np.asarray(states_action, np.float32)[c * cfg.EL + order].T)
        in_maps.append({
            "ls0": ls0,
            "idx": pre["idx"][c],
            "corr": pre["corr"][c],
            "wpack": wpack,
        })
    return in_maps


def readout_host(cfg, pre, ls_outs, states_graph_ids, num_graphs,
                 Wr1, br1, Wr2, br2, Wr3, br3):
    E, EL = cfg.E, cfg.EL
    ls_full = np.empty((E, 32), np.float32)
    for c in range(cfg.NC):
        order = pre["orders"][c]
        ls_full[c * EL + order] = ls_outs[c].T
    gids = np.asarray(states_graph_ids, np.int64)
    gs = np.zeros((num_graphs, 32), np.float32)
    np.add.at(gs, gids, ls_full)
    h = _selu_np(gs @ np.asarray(Wr1, np.float32) + np.asarray(br1, np.float32))
    h = _selu_np(h @ np.asarray(Wr2, np.float32) + np.asarray(br2, np.float32))
    a = np.maximum(h @ np.asarray(Wr3, np.float32)
                   + np.asarray(br3, np.float32), 0.0)
    return a.astype(np.float32)


_BUILD_CACHE = {}


def run_device(cfg, pre, in_maps, use_sim=False, trace=False):
    key = (cfg.E, cfg.T, tuple(pre["C_r"]))
    if key not in _BUILD_CACHE:
        _BUILD_CACHE[key] = build_full(cfg, pre["C_r"], pre["CTOT"])
    nc = _BUILD_CACHE[key]
    if use_sim:
        from concourse.bass_interp import MultiCoreSim
        sim = MultiCoreSim(nc, num_cores=cfg.NC)
        for c in range(cfg.NC):
            for k, v in in_maps[c].items():
                sim.cores[c].tensor(k)[:] = v
        sim.simulate()
        outs = [np.array(sim.cores[c].mem_tensor("ls_out"))
                for c in range(cfg.NC)]
        return outs, None
    res = run_bass_kernel_spmd(nc, in_maps, core_ids=list(range(cfg.NC)),
                               trace=trace)
    outs = [res.results[c]["ls_out"] for c in range(cfg.NC)]
    return outs, res


def _kernel_impl(inputs, use_sim=False, T=8, num_graphs=64, trace=False):
    states_action = np.asarray(inputs["states_action"], np.float32)
    E = states_action.shape[0]
    cfg = Cfg(E=E, T=T, n_cores=8)
    pre = preprocess(cfg, inputs["states_first"], inputs["states_second"])
    in_maps = make_in_maps(cfg, pre, states_action, inputs["Wm"],
                           inputs["bm"], inputs["W"], inputs["U"], inputs["b"])
    ls_outs, res = run_device(cfg, pre, in_maps, use_sim=use_sim, trace=trace)
    a = readout_host(cfg, pre, ls_outs, inputs["states_graph_ids"], num_graphs,
                     inputs["Wr1"], inputs["br1"], inputs["Wr2"],
                     inputs["br2"], inputs["Wr3"], inputs["br3"])
    return a, res


def kernel(**inputs):
    a, _ = _kernel_impl(inputs)
    return a



# revision 15
# speedup vs baseline: 1.1174x; 1.0076x over previous
"""Trainium2 Bass kernel for GNN message passing (nn_Actor_71141838291282).

Algorithm (per message-passing iteration, T=8):
    msg  = selu(ls[first] @ Wm1 + ls[second] @ Wm2 + bm)     [M, 32]
    agg  = segment_sum(msg, second, E)                        [E, 32]
    ls   = GRU(agg, ls)                                       [E, 32]
Readout: graph segment-sum + 3-layer MLP (done host-side; negligible work).

Distribution: 8 NeuronCores, shard by DESTINATION node (states_second).
Core c owns dests [c*EL, (c+1)*EL), EL = E/8 = 32768.
Per iteration each core:
  1. A = ls_loc @ Wm1 + bm (fp16, node-major) -> DRAM;  B = ls_loc @ Wm2 (fp16, SBUF)
  2. AllGather A -> full table A_ext [E, 32] fp16 in local DRAM
  3. "Rounds": dests per core are relabeled by descending in-degree (host-side
     permutation), so round r = the r-th edge of every dest with degree > r is a
     contiguous PREFIX of the dest space.  For each round: prefill a buffer with
     B[dest], indirect-DMA gather-add A_ext[first] (cce add) into it, apply SELU
     (exp(min(x,0)) composite, no selects), accumulate into agg (f32).
     Pad slots gather a -3e4 pad row => selu == -lam*alpha exactly; corrected by
     initializing agg with +lam*alpha*npad (static).
  4. GRU feat-major via PE matmuls (W/U stationary) + DVE/ACT elementwise.

All indices/permutations are computed host-side in numpy (static data).
"""

import math
import numpy as np

import concourse.bass as bass
import concourse.mybir as mybir
import concourse.tile as tile
from concourse import bacc
from concourse.bass_utils import run_bass_kernel_spmd

F32 = mybir.dt.float32
F16 = mybir.dt.float16
I32 = mybir.dt.int32

LAM = 1.0507009873554805
ALPHA = 1.6732632423543772
PAD_VAL = -30000.0  # pad row value in A table (f16 range)

P = 128
# NOTE: the HW DGE reads ONE index per partition per indirect DMA and
# fetches contiguous rows for multi-column offset APs, so batching
# columns into one instruction is NOT possible (verified by probe).
GATHER_BATCH = 1  # offset-AP columns per indirect DMA instruction


class Cfg:
    def __init__(self, E=262144, T=8, n_cores=8):
        self.E = E
        self.T = T
        self.NC = n_cores
        self.EL = E // n_cores
        self.C = self.EL // P            # grid cols (dest rank i at (i%P, i//P))
        assert self.EL % P == 0


# ---------------------------------------------------------------------------
# Host preprocessing
# ---------------------------------------------------------------------------

def preprocess(cfg, states_first, states_second):
    """Build per-core static index data.

    Returns dict with:
      order[c]   : [EL]   local node id for dest rank i  (descending degree)
      gperm      : [E]    global A-table row for global node u
      idx[c]     : [P, CTOT] int32 gather rows per round (concatenated cols)
      corr[c]    : [P, C] f32  = +LAM*ALPHA * npad  (agg init)
      C_r        : list of per-round col counts (compile-time consts)
    """
    E, EL, NC, C = cfg.E, cfg.EL, cfg.NC, cfg.C
    first = np.asarray(states_first, dtype=np.int64)
    second = np.asarray(states_second, dtype=np.int64)

    core_of = second // EL
    orders = []
    ranks = np.empty(E, dtype=np.int64)
    degs_sorted = []
    per_core_edges = []
    for c in range(NC):
        m = core_of == c
        ef = first[m]
        es = second[m] - c * EL
        deg = np.bincount(es, minlength=EL)
        order = np.argsort(-deg, kind="stable")        # rank -> local id
        rank = np.empty(EL, dtype=np.int64)
        rank[order] = np.arange(EL)
        orders.append(order)
        ranks[c * EL:(c + 1) * EL] = rank
        degs_sorted.append(deg[order])                  # descending
        per_core_edges.append((ef, es, rank))

    # global A-table row of node u.  Layout: [half][core][rank % (EL/2)] so
    # each half of the table is AllGather-able independently (half h of the
    # table = concat over cores of their local ranks [h*EL/2, (h+1)*EL/2)).
    H = EL // 2
    cores = np.arange(E) // EL
    halves = ranks // H
    gperm = halves * (E // 2) + cores * H + (ranks % H)

    maxdeg = max(int(d[0]) for d in degs_sorted)
    R = maxdeg
    # per-round edge counts n_r per core; global padded col counts
    C_r = []
    for r in range(R):
        n_r_max = max(int(np.count_nonzero(d > r)) for d in degs_sorted)
        C_r.append(max(1, math.ceil(n_r_max / P)))
    CTOT = sum(C_r)

    idx_all = []
    corr_all = []
    for c in range(NC):
        ef, es, rank = per_core_edges[c]
        d_sorted = degs_sorted[c]
        # sort edges by (dest rank, arbitrary); round index = occurrence count
        dest_rank = rank[es]
        o = np.argsort(dest_rank, kind="stable")
        dr = dest_rank[o]
        rows = gperm[ef[o]]
        # occurrence number within each dest
        occ = np.arange(len(dr)) - np.concatenate(
            ([0], np.cumsum(np.bincount(dr, minlength=EL))))[dr]
        idx = np.full((R, EL), cfg.E, dtype=np.int64)   # pad row = E
        idx[occ, dr] = rows
        # build [P, CTOT] layout: round r slots i in [0, P*C_r[r]), slot i=(p+P*cc)
        cols = np.zeros((P, CTOT), dtype=np.int32)
        off = 0
        npad = np.zeros(EL, dtype=np.int64)
        for r in range(R):
            ncols = C_r[r]
            sl = idx[r, :P * ncols]                       # slot i -> row
            pads = sl == cfg.E
            npad[:P * ncols] += pads
            cols[:, off:off + ncols] = sl.reshape(ncols, P).T
            off += ncols
        idx_all.append(cols)
        corr = (LAM * ALPHA) * npad.astype(np.float32)
        corr_all.append(corr.reshape(C, P).T.copy())     # [P, C]

    return dict(orders=orders, gperm=gperm, idx=idx_all, corr=corr_all,
                C_r=C_r, R=R, CTOT=CTOT)


# ---------------------------------------------------------------------------
# Device kernel builder
# ---------------------------------------------------------------------------

def build_full(cfg, C_r, CTOT):
    """Build the complete SPMD graph (all 8 cores run this identically).

    Column-block-major schedule: phase 3 (gather rounds) runs per block of
    W grid columns; as soon as a block's agg is final, its GRU chunks and
    the NEXT iteration's A/B matmuls for those nodes run — pipelined under
    the (GpSimd-bound) gathers of the remaining blocks.  The AllGather is
    the only global barrier per iteration.
    """
    E, EL, NC, C, T = cfg.E, cfg.EL, cfg.NC, cfg.C, cfg.T
    R = len(C_r)
    nc = bacc.Bacc("TRN2", target_bir_lowering=False, debug=False,
                   num_devices=NC)

    ls0 = nc.dram_tensor("ls0", [32, EL], F32, kind="ExternalInput")
    idx_in = nc.dram_tensor("idx", [P, CTOT], I32, kind="ExternalInput")
    corr_in = nc.dram_tensor("corr", [P, C], F32, kind="ExternalInput")
    # packed weights: wm1[0:32,0:32] wm2[0:32,32:64] bm[0:32,64] w[0:32,65:161]
    # u[0:32,161:257] b0[0:96,257] b1[0:96,258]
    wp_in = nc.dram_tensor("wpack", [96, 260], F32, kind="ExternalInput")
    ls_out = nc.dram_tensor("ls_out", [32, EL], F32, kind="ExternalOutput")

    a_loc = nc.dram_tensor("a_loc", [EL, 32], F16)
    # double-buffered table: AG for iteration t+1 overlaps t's gathers
    a_exts = [nc.dram_tensor(f"a_ext{i}", [E + P, 32], F16,
                             addr_space="Shared") for i in range(2)]
    ls_ping = nc.dram_tensor("ls_ping", [32, EL], F32)
    ls_pong = nc.dram_tensor("ls_pong", [32, EL], F32)

    CH = min(2048, EL)       # chunk (free dim) for A/B and GRU phases
    NMM = min(512, CH)       # matmul free dim
    n_ch = EL // CH
    GCOL = CH // P           # grid cols per chunk
    W = min(32, C)           # grid cols per phase-3 block
    NB = C // W              # number of blocks
    CPB = max(1, W // GCOL)  # GRU chunks per block
    assert C % W == 0 and W % GCOL == 0

    # per-round column offsets into the idx layout
    off_r = [0]
    for r in range(R):
        off_r.append(off_r[-1] + C_r[r])

    AF = mybir.ActivationFunctionType
    ALU = mybir.AluOpType

    with tile.TileContext(nc) as tc:
        with (
            tc.tile_pool(name="sb", bufs=1) as sb,
            tc.tile_pool(name="io", bufs=2) as io,
            tc.tile_pool(name="ab", bufs=2) as ab,
            tc.tile_pool(name="rp", bufs=3) as rp,
            tc.tile_pool(name="ag", bufs=2) as agp,
            tc.tile_pool(name="mm", bufs=2, space="PSUM") as pmm,
        ):
            # ---- persistent SBUF ----
            idx_sb = sb.tile([P, CTOT], I32, tag="idx")
            nc.sync.dma_start(idx_sb[:], idx_in[:, :])
            corr_sb = sb.tile([P, C], F32, tag="corr")
            nc.sync.dma_start(corr_sb[:], corr_in[:, :])
            wp = sb.tile([96, 260], F32, tag="wp")
            nc.sync.dma_start(wp[:], wp_in[:, :])
            wm1 = wp[0:32, 0:32]
            wm2 = wp[0:32, 32:64]
            bm_ap = wp[0:32, 64:65]
            w_ap = wp[0:32, 65:161]
            u_ap = wp[0:32, 161:257]
            b0_ap = wp[0:96, 257:258]
            b1_ap = wp[0:96, 258:259]

            b_nm = sb.tile([P, C, 32], F16, tag="b_nm")

            padrow = sb.tile([P, 32], F16, tag="padrow")
            nc.vector.memset(padrow[:], PAD_VAL)
            for ae in a_exts:
                nc.sync.dma_start(ae[E:E + P, :], padrow[:])

            a_loc3 = a_loc[:, :].rearrange("(c p) f -> p c f", p=P)
            HL = EL // 2
            mid_ch = max(1, n_ch // 2)

            def allgather_half(half, dst_ext):
                nc.gpsimd.collective_compute(
                    "AllGather", ALU.bypass,
                    replica_groups=[list(range(NC))],
                    ins=[a_loc[half * HL:(half + 1) * HL, :].opt()],
                    outs=[dst_ext[half * (E // 2):
                                  (half + 1) * (E // 2), :].opt()],
                )

            def ls_src(t):
                if t == 0:
                    return ls0
                return ls_ping if t % 2 == 1 else ls_pong

            def ls_dst(t):
                if t == cfg.T - 1:
                    return ls_out
                return ls_ping if t % 2 == 0 else ls_pong

            def phase1_chunk(lsc, ch):
                """A/B for node chunk ch from feat-major ls tile lsc."""
                pa = pmm.tile([32, CH], F32, tag="mm", space="PSUM")
                pb = pmm.tile([32, CH], F32, tag="mm", space="PSUM")
                for k in range(CH // NMM):
                    nc.tensor.matmul(pa[:, k * NMM:(k + 1) * NMM],
                                     wm1, lsc[:, k * NMM:(k + 1) * NMM])
                for k in range(CH // NMM):
                    nc.tensor.matmul(pb[:, k * NMM:(k + 1) * NMM],
                                     wm2, lsc[:, k * NMM:(k + 1) * NMM])
                aT = ab.tile([32, CH], F16, tag="aT")
                bT = ab.tile([32, CH], F16, tag="bT")
                nc.scalar.activation(aT[:], pa[:], AF.Identity, bias=bm_ap)
                nc.scalar.activation(bT[:], pb[:], AF.Identity)
                # 32x32 block transpose + block remap into node-major grid
                # tmp[q, 32k+f] = srcT[f, 32k+q];  node = ch*CH + 32k + q;
                # grid (p, c) = (32*(k%4)+q, ch*GCOL + k//4)
                a_stg = ab.tile([P, GCOL, 32], F16, tag="a_stg")
                for srcT, dstG, c0 in ((aT, a_stg, 0),
                                       (bT, b_nm, ch * GCOL)):
                    tT = ab.tile([32, CH], F16, tag="tT")
                    nc.vector.transpose(tT[:], srcT[:])
                    t3 = tT[:].rearrange("q (k f) -> q k f", f=32)
                    for p32 in range(4):
                        nc.vector.tensor_copy(
                            dstG[32 * p32:32 * (p32 + 1),
                                 c0:c0 + GCOL, :],
                            t3[:, p32::4, :])
                # A chunk -> DRAM table rows (row i = p + P*c)
                nc.sync.dma_start(
                    a_loc3[:, ch * GCOL:(ch + 1) * GCOL, :], a_stg[:])

            def phase4_chunk(agg_ap, ch, src, dst):
                """GRU for node chunk ch; returns feat-major ls' tile."""
                o = ch * CH
                agT = io.tile([32, CH], F32, tag="agT")
                tmp2 = io.tile([P, GCOL * 32], F32, tag="tmp2")
                nc.vector.transpose(
                    tmp2[:], agg_ap.rearrange("p a b -> p (a b)"))
                tmp23 = tmp2[:].rearrange("p (c q) -> p c q", q=32)
                agT3 = agT[:].rearrange("f (c w) -> f c w", w=P)
                for p32 in range(4):
                    nc.vector.tensor_copy(
                        agT3[:, :, 32 * p32:32 * (p32 + 1)],
                        tmp23[32 * p32:32 * (p32 + 1), :, :])
                lsc = io.tile([32, CH], F32, tag="lsio")
                nc.sync.dma_start(lsc[:], src[:, o:o + CH])
                pxm = pmm.tile([96, CH], F32, tag="mm", space="PSUM")
                phm = pmm.tile([96, CH], F32, tag="mm", space="PSUM")
                for k in range(CH // NMM):
                    nc.tensor.matmul(pxm[:, k * NMM:(k + 1) * NMM],
                                     w_ap, agT[:, k * NMM:(k + 1) * NMM])
                for k in range(CH // NMM):
                    nc.tensor.matmul(phm[:, k * NMM:(k + 1) * NMM],
                                     u_ap, lsc[:, k * NMM:(k + 1) * NMM])
                # TT requires equal base partitions on both SB inputs;
                # the schedule below realigns operands via ACT placement.
                xm = io.tile([96, CH], F32, tag="xm")
                hm = io.tile([96, CH], F32, tag="hm")
                sc = ab.tile([32, CH], F32, tag="sc")
                nc.scalar.activation(xm[:], pxm[:], AF.Identity, bias=b0_ap)
                nc.scalar.activation(hm[:], phm[:], AF.Identity, bias=b1_ap)
                # t1: xm[0:64] += hm[0:64]  (z and r pre-activations)
                nc.vector.tensor_tensor(xm[0:64, :], xm[0:64, :],
                                        hm[0:64, :], ALU.add)
                # z@xm[0:32], r@xm[32:64]
                nc.scalar.activation(xm[0:64, :], xm[0:64, :], AF.Sigmoid)
                # realign hh to base 32 (hm[0:64] is dead now)
                nc.scalar.activation(hm[32:64, :], hm[64:96, :], AF.Identity)
                # rh = r*hh -> hm[64:96] (base-64 for the t2 add)
                nc.vector.tensor_tensor(hm[64:96, :], xm[32:64, :],
                                        hm[32:64, :], ALU.mult)
                # t2 = xh + rh -> xm[64:96]
                nc.vector.tensor_tensor(xm[64:96, :], xm[64:96, :],
                                        hm[64:96, :], ALU.add)
                # cand -> hm[0:32] (base 0)
                nc.scalar.activation(hm[0:32, :], xm[64:96, :], AF.Tanh)
                # dd = ls - cand -> sc;  e2 = z*dd -> sc
                nc.vector.tensor_tensor(sc[:], lsc[:], hm[0:32, :],
                                        ALU.subtract)
                nc.vector.tensor_tensor(sc[:], xm[0:32, :], sc[:], ALU.mult)
                # ls' = cand + e2 -> lsc
                nc.vector.tensor_tensor(lsc[:], hm[0:32, :], sc[:], ALU.add)
                nc.sync.dma_start(dst[:, o:o + CH], lsc[:])
                return lsc

            # ---- t=0 A/B from ls0 (AG half 0 fires mid-way) ----
            for ch in range(n_ch):
                lsc = io.tile([32, CH], F32, tag="lsio")
                nc.sync.dma_start(lsc[:], ls0[:, ch * CH:(ch + 1) * CH])
                phase1_chunk(lsc, ch)
                if ch + 1 == mid_ch:
                    allgather_half(0, a_exts[0])
            allgather_half(1, a_exts[0])

            for t in range(cfg.T):
                src = ls_src(t)
                dst = ls_dst(t)
                a_ext = a_exts[t % 2]
                ae_next = a_exts[(t + 1) % 2]
                done_ch = 0

                for cb in range(NB):
                    cb0 = cb * W
                    # ---- phase 3: gather rounds for this column block ----
                    agg = agp.tile([P, W, 32], F32, tag="agg")
                    nc.vector.tensor_copy(
                        agg[:], corr_sb[:, cb0:cb0 + W, None]
                        .to_broadcast([P, W, 32]))
                    for r in range(R):
                        c_hi = min(C_r[r], cb0 + W)
                        if c_hi <= cb0:
                            break          # C_r is non-increasing
                        cw = c_hi - cb0
                        rb = rp.tile([P, W, 32], F16, tag="rb")
                        rbv = rb[:, 0:cw, :]
                        rbf = rb[:].rearrange("p a b -> p (a b)")
                        nc.vector.tensor_copy(rbv, b_nm[:, cb0:c_hi, :])
                        # indirect gather: one column per instruction (the
                        # HW DGE supports only one offset per partition)
                        for cc in range(cw):
                            col = off_r[r] + cb0 + cc
                            nc.gpsimd.indirect_dma_start(
                                out=rbf[:, cc * 32:(cc + 1) * 32],
                                out_offset=None,
                                in_=a_ext[:, :],
                                in_offset=bass.IndirectOffsetOnAxis(
                                    ap=idx_sb[:, col:col + 1], axis=0),
                                compute_op=ALU.add,
                            )
                        # selu: m=min(x,0); v=LAM*max(x,0) (in-place rb);
                        # e=exp(m); s=LAM*ALPHA*e-LAM*ALPHA; s+=v; agg+=s
                        mt = rp.tile([P, W, 32], F16, tag="mt")
                        mtv = mt[:, 0:cw, :]
                        nc.vector.tensor_scalar(mtv, rbv, 0.0, None, ALU.min)
                        nc.vector.tensor_scalar(rbv, rbv, 0.0, LAM, ALU.max,
                                                ALU.mult)
                        nc.scalar.activation(mtv, mtv, AF.Exp)
                        nc.vector.tensor_scalar(mtv, mtv, LAM * ALPHA,
                                                -LAM * ALPHA, ALU.mult,
                                                ALU.add)
                        nc.vector.tensor_tensor(mtv, mtv, rbv, ALU.add)
                        av = agg[:, 0:cw, :]
                        nc.vector.tensor_tensor(av, av, mtv, ALU.add)

                    # ---- phase 4 (+ next iteration's A/B) per chunk ----
                    for j in range(CPB):
                        ch = cb * CPB + j
                        lsc = phase4_chunk(
                            agg[:, j * GCOL:(j + 1) * GCOL, :], ch, src, dst)
                        if t < cfg.T - 1:
                            phase1_chunk(lsc, ch)
                            done_ch += 1
                            if done_ch == mid_ch:
                                allgather_half(0, ae_next)
                if t < cfg.T - 1:
                    allgather_half(1, ae_next)

    nc.compile()
    return nc


# ---------------------------------------------------------------------------
# Host-side glue
# ---------------------------------------------------------------------------

def _selu_np(x):
    return (LAM * (np.maximum(x, 0.0)
            + ALPHA * (np.expm1(np.minimum(x, 0.0))))).astype(np.float32)


def make_in_maps(cfg, pre, states_action, Wm, bm, W, U, b):
    Wm = np.asarray(Wm, np.float32)
    wpack = np.zeros((96, 260), np.float32)
    wpack[0:32, 0:32] = Wm[:32]
    wpack[0:32, 32:64] = Wm[32:]
    wpack[0:32, 64] = np.asarray(bm, np.float32)
    wpack[0:32, 65:161] = np.asarray(W, np.float32)
    wpack[0:32, 161:257] = np.asarray(U, np.float32)
    wpack[0:96, 257] = np.asarray(b[0], np.float32)
    wpack[0:96, 258] = np.asarray(b[1], np.float32)
    in_maps = []
    for c in range(cfg.NC):
        order = pre["orders"][c]
        ls0 = np.ascontiguousarray(
            np.asarray(states_action, np.float32)[c * cfg.EL + order].T)
        in_maps.append({
            "ls0": ls0,
            "idx": pre["idx"][c],
            "corr": pre["corr"][c],
            "wpack": wpack,
        })
    return in_maps


def readout_host(cfg, pre, ls_outs, states_graph_ids, num_graphs,
                 Wr1, br1, Wr2, br2, Wr3, br3):
    E, EL = cfg.E, cfg.EL
    ls_full = np.empty((E, 32), np.float32)
    for c in range(cfg.NC):
        order = pre["orders"][c]
        ls_full[c * EL + order] = ls_outs[c].T
    gids = np.asarray(states_graph_ids, np.int64)
    gs = np.zeros((num_graphs, 32), np.float32)
    np.add.at(gs, gids, ls_full)
    h = _selu_np(gs @ np.asarray(Wr1, np.float32) + np.asarray(br1, np.float32))
    h = _selu_np(h @ np.asarray(Wr2, np.float32) + np.asarray(br2, np.float32))
    a = np.maximum(h @ np.asarray(Wr3, np.float32)
                   + np.asarray(br3, np.float32), 0.0)
    return a.astype(np.float32)


_BUILD_CACHE = {}


def run_device(cfg, pre, in_maps, use_sim=False, trace=False):
    key = (cfg.E, cfg.T, tuple(pre["C_r"]))
    if key not in _BUILD_CACHE:
        _BUILD_CACHE[key] = build_full(cfg, pre["C_r"], pre["CTOT"])
    nc = _BUILD_CACHE[key]
    if use_sim:
        from concourse.bass_interp import MultiCoreSim
        sim = MultiCoreSim(nc, num_cores=cfg.NC)
        for c in range(cfg.NC):
            for k, v in in_maps[c].items():
                sim.cores[c].tensor(k)[:] = v
        sim.simulate()
        outs = [np.array(sim.cores[c].mem_tensor("ls_out"))
                for c in range(cfg.NC)]
        return outs, None
    res = run_bass_kernel_spmd(nc, in_maps, core_ids=list(range(cfg.NC)),
                               trace=trace)
    outs = [res.results[c]["ls_out"] for c in range(cfg.NC)]
    return outs, res


def _kernel_impl(inputs, use_sim=False, T=8, num_graphs=64, trace=False):
    states_action = np.asarray(inputs["states_action"], np.float32)
    E = states_action.shape[0]
    cfg = Cfg(E=E, T=T, n_cores=8)
    pre = preprocess(cfg, inputs["states_first"], inputs["states_second"])
    in_maps = make_in_maps(cfg, pre, states_action, inputs["Wm"],
                           inputs["bm"], inputs["W"], inputs["U"], inputs["b"])
    ls_outs, res = run_device(cfg, pre, in_maps, use_sim=use_sim, trace=trace)
    a = readout_host(cfg, pre, ls_outs, inputs["states_graph_ids"], num_graphs,
                     inputs["Wr1"], inputs["br1"], inputs["Wr2"],
                     inputs["br2"], inputs["Wr3"], inputs["br3"])
    return a, res


def kernel(**inputs):
    a, _ = _kernel_impl(inputs)
    return a



# revision 21
# speedup vs baseline: 1.1259x; 1.0077x over previous
"""Trainium2 Bass kernel for GNN message passing (nn_Actor_71141838291282).

Algorithm (per message-passing iteration, T=8):
    msg  = selu(ls[first] @ Wm1 + ls[second] @ Wm2 + bm)     [M, 32]
    agg  = segment_sum(msg, second, E)                        [E, 32]
    ls   = GRU(agg, ls)                                       [E, 32]
Readout: graph segment-sum + 3-layer MLP (done host-side; negligible work).

Distribution: 8 NeuronCores, shard by DESTINATION node (states_second).
Core c owns dests [c*EL, (c+1)*EL), EL = E/8 = 32768.
Per iteration each core:
  1. A = ls_loc @ Wm1 + bm (fp16, node-major) -> DRAM;  B = ls_loc @ Wm2 (fp16, SBUF)
  2. AllGather A -> full table A_ext [E, 32] fp16 in local DRAM
  3. "Rounds": dests per core are relabeled by descending in-degree (host-side
     permutation), so round r = the r-th edge of every dest with degree > r is a
     contiguous PREFIX of the dest space.  For each round: prefill a buffer with
     B[dest], indirect-DMA gather-add A_ext[first] (cce add) into it, apply SELU
     (exp(min(x,0)) composite, no selects), accumulate into agg (f32).
     Pad slots gather a -3e4 pad row => selu == -lam*alpha exactly; corrected by
     initializing agg with +lam*alpha*npad (static).
  4. GRU feat-major via PE matmuls (W/U stationary) + DVE/ACT elementwise.

All indices/permutations are computed host-side in numpy (static data).
"""

import math
import numpy as np

import concourse.bass as bass
import concourse.mybir as mybir
import concourse.tile as tile
from concourse import bacc
from concourse.bass_utils import run_bass_kernel_spmd

F32 = mybir.dt.float32
F16 = mybir.dt.float16
I32 = mybir.dt.int32

LAM = 1.0507009873554805
ALPHA = 1.6732632423543772
PAD_VAL = -30000.0  # pad row value in A table (f16 range)

P = 128
# NOTE: the HW DGE reads ONE index per partition per indirect DMA and
# fetches contiguous rows for multi-column offset APs, so batching
# columns into one instruction is NOT possible (verified by probe).
GATHER_BATCH = 1  # offset-AP columns per indirect DMA instruction


class Cfg:
    def __init__(self, E=262144, T=8, n_cores=8):
        self.E = E
        self.T = T
        self.NC = n_cores
        self.EL = E // n_cores
        self.C = self.EL // P            # grid cols (dest rank i at (i%P, i//P))
        assert self.EL % P == 0


# ---------------------------------------------------------------------------
# Host preprocessing
# ---------------------------------------------------------------------------

def preprocess(cfg, states_first, states_second):
    """Build per-core static index data.

    Returns dict with:
      order[c]   : [EL]   local node id for dest rank i  (descending degree)
      gperm      : [E]    global A-table row for global node u
      idx[c]     : [P, CTOT] int32 gather rows per round (concatenated cols)
      corr[c]    : [P, C] f32  = +LAM*ALPHA * npad  (agg init)
      C_r        : list of per-round col counts (compile-time consts)
    """
    E, EL, NC, C = cfg.E, cfg.EL, cfg.NC, cfg.C
    first = np.asarray(states_first, dtype=np.int64)
    second = np.asarray(states_second, dtype=np.int64)

    core_of = second // EL
    orders = []
    ranks = np.empty(E, dtype=np.int64)
    degs_sorted = []
    per_core_edges = []
    for c in range(NC):
        m = core_of == c
        ef = first[m]
        es = second[m] - c * EL
        deg = np.bincount(es, minlength=EL)
        order = np.argsort(-deg, kind="stable")        # rank -> local id
        rank = np.empty(EL, dtype=np.int64)
        rank[order] = np.arange(EL)
        orders.append(order)
        ranks[c * EL:(c + 1) * EL] = rank
        degs_sorted.append(deg[order])                  # descending
        per_core_edges.append((ef, es, rank))

    # global A-table row of node u.  Layout: [quarter][core][rank % (EL/4)]
    # so each quarter of the table is AllGather-able independently (quarter
    # q = concat over cores of their local ranks [q*EL/4, (q+1)*EL/4)).
    Q = EL // 4
    cores = np.arange(E) // EL
    quarters = ranks // Q
    gperm = quarters * (E // 4) + cores * Q + (ranks % Q)

    maxdeg = max(int(d[0]) for d in degs_sorted)
    R = maxdeg
    # per-round edge counts n_r per core; global padded col counts
    C_r = []
    for r in range(R):
        n_r_max = max(int(np.count_nonzero(d > r)) for d in degs_sorted)
        C_r.append(max(1, math.ceil(n_r_max / P)))
    CTOT = sum(C_r)

    idx_all = []
    corr_all = []
    for c in range(NC):
        ef, es, rank = per_core_edges[c]
        d_sorted = degs_sorted[c]
        # sort edges by (dest rank, arbitrary); round index = occurrence count
        dest_rank = rank[es]
        o = np.argsort(dest_rank, kind="stable")
        dr = dest_rank[o]
        rows = gperm[ef[o]]
        # occurrence number within each dest
        occ = np.arange(len(dr)) - np.concatenate(
            ([0], np.cumsum(np.bincount(dr, minlength=EL))))[dr]
        idx = np.full((R, EL), cfg.E, dtype=np.int64)   # pad row = E
        idx[occ, dr] = rows
        # build [P, CTOT] layout: round r slots i in [0, P*C_r[r]), slot i=(p+P*cc)
        cols = np.zeros((P, CTOT), dtype=np.int32)
        off = 0
        npad = np.zeros(EL, dtype=np.int64)
        for r in range(R):
            ncols = C_r[r]
            sl = idx[r, :P * ncols]                       # slot i -> row
            pads = sl == cfg.E
            npad[:P * ncols] += pads
            cols[:, off:off + ncols] = sl.reshape(ncols, P).T
            off += ncols
        idx_all.append(cols)
        corr = (LAM * ALPHA) * npad.astype(np.float32)
        corr_all.append(corr.reshape(C, P).T.copy())     # [P, C]

    return dict(orders=orders, gperm=gperm, idx=idx_all, corr=corr_all,
                C_r=C_r, R=R, CTOT=CTOT)


# ---------------------------------------------------------------------------
# Device kernel builder
# ---------------------------------------------------------------------------

def build_full(cfg, C_r, CTOT):
    """Build the complete SPMD graph (all 8 cores run this identically).

    Column-block-major schedule: phase 3 (gather rounds) runs per block of
    W grid columns; as soon as a block's agg is final, its GRU chunks and
    the NEXT iteration's A/B matmuls for those nodes run — pipelined under
    the (GpSimd-bound) gathers of the remaining blocks.  The AllGather is
    the only global barrier per iteration.
    """
    E, EL, NC, C, T = cfg.E, cfg.EL, cfg.NC, cfg.C, cfg.T
    R = len(C_r)
    nc = bacc.Bacc("TRN2", target_bir_lowering=False, debug=False,
                   num_devices=NC)

    ls0 = nc.dram_tensor("ls0", [32, EL], F32, kind="ExternalInput")
    idx_in = nc.dram_tensor("idx", [P, CTOT], I32, kind="ExternalInput")
    corr_in = nc.dram_tensor("corr", [P, C], F32, kind="ExternalInput")
    # packed weights: wm1[0:32,0:32] wm2[0:32,32:64] bm[0:32,64] w[0:32,65:161]
    # u[0:32,161:257] b0[0:96,257] b1[0:96,258]
    wp_in = nc.dram_tensor("wpack", [96, 260], F32, kind="ExternalInput")
    ls_out = nc.dram_tensor("ls_out", [32, EL], F32, kind="ExternalOutput")

    a_loc = nc.dram_tensor("a_loc", [EL, 32], F16)
    # double-buffered table: AG for iteration t+1 overlaps t's gathers
    a_exts = [nc.dram_tensor(f"a_ext{i}", [E + P, 32], F16,
                             addr_space="Shared") for i in range(2)]
    ls_ping = nc.dram_tensor("ls_ping", [32, EL], F32)
    ls_pong = nc.dram_tensor("ls_pong", [32, EL], F32)

    CH = min(2048, EL)       # chunk (free dim) for A/B and GRU phases
    NMM = min(512, CH)       # matmul free dim
    n_ch = EL // CH
    GCOL = CH // P           # grid cols per chunk
    W = min(32, C)           # grid cols per phase-3 block
    NB = C // W              # number of blocks
    CPB = max(1, W // GCOL)  # GRU chunks per block
    assert C % W == 0 and W % GCOL == 0

    # per-round column offsets into the idx layout
    off_r = [0]
    for r in range(R):
        off_r.append(off_r[-1] + C_r[r])

    AF = mybir.ActivationFunctionType
    ALU = mybir.AluOpType

    with tile.TileContext(nc) as tc:
        with (
            tc.tile_pool(name="sb", bufs=1) as sb,
            tc.tile_pool(name="io", bufs=2) as io,
            tc.tile_pool(name="ab", bufs=2) as ab,
            tc.tile_pool(name="rp", bufs=4) as rp,
            tc.tile_pool(name="ag", bufs=2) as agp,
            tc.tile_pool(name="mm", bufs=2, space="PSUM") as pmm,
        ):
            # ---- persistent SBUF ----
            idx_sb = sb.tile([P, CTOT], I32, tag="idx")
            nc.sync.dma_start(idx_sb[:], idx_in[:, :])
            corr_sb = sb.tile([P, C], F32, tag="corr")
            nc.sync.dma_start(corr_sb[:], corr_in[:, :])
            wp = sb.tile([96, 260], F32, tag="wp")
            nc.sync.dma_start(wp[:], wp_in[:, :])
            wm1 = wp[0:32, 0:32]
            wm2 = wp[0:32, 32:64]
            bm_ap = wp[0:32, 64:65]
            w_ap = wp[0:32, 65:161]
            u_ap = wp[0:32, 161:257]
            b0_ap = wp[0:96, 257:258]
            b1_ap = wp[0:96, 258:259]

            b_nm = sb.tile([P, C, 32], F16, tag="b_nm")

            padrow = sb.tile([P, 32], F16, tag="padrow")
            nc.vector.memset(padrow[:], PAD_VAL)
            for ae in a_exts:
                nc.sync.dma_start(ae[E:E + P, :], padrow[:])

            a_loc3 = a_loc[:, :].rearrange("(c p) f -> p c f", p=P)
            QL = EL // 4
            # fire AG part q once ceil(n_ch*(q+1)/4) chunks are written
            ag_thresh = [math.ceil(n_ch * (q + 1) / 4) for q in range(3)]

            def allgather_part(q, dst_ext):
                nc.gpsimd.collective_compute(
                    "AllGather", ALU.bypass,
                    replica_groups=[list(range(NC))],
                    ins=[a_loc[q * QL:(q + 1) * QL, :].opt()],
                    outs=[dst_ext[q * (E // 4):
                                  (q + 1) * (E // 4), :].opt()],
                )

            def ls_src(t):
                if t == 0:
                    return ls0
                return ls_ping if t % 2 == 1 else ls_pong

            def ls_dst(t):
                if t == cfg.T - 1:
                    return ls_out
                return ls_ping if t % 2 == 0 else ls_pong

            def phase1_chunk(lsc, ch):
                """A/B for node chunk ch from feat-major ls tile lsc."""
                pa = pmm.tile([32, CH], F32, tag="mm", space="PSUM")
                pb = pmm.tile([32, CH], F32, tag="mm", space="PSUM")
                for k in range(CH // NMM):
                    nc.tensor.matmul(pa[:, k * NMM:(k + 1) * NMM],
                                     wm1, lsc[:, k * NMM:(k + 1) * NMM])
                for k in range(CH // NMM):
                    nc.tensor.matmul(pb[:, k * NMM:(k + 1) * NMM],
                                     wm2, lsc[:, k * NMM:(k + 1) * NMM])
                aT = ab.tile([32, CH], F16, tag="aT")
                bT = ab.tile([32, CH], F16, tag="bT")
                nc.scalar.activation(aT[:], pa[:], AF.Identity, bias=bm_ap)
                nc.scalar.activation(bT[:], pb[:], AF.Identity)
                # 32x32 block transpose + block remap into node-major grid
                # tmp[q, 32k+f] = srcT[f, 32k+q];  node = ch*CH + 32k + q;
                # grid (p, c) = (32*(k%4)+q, ch*GCOL + k//4)
                a_stg = ab.tile([P, GCOL, 32], F16, tag="a_stg")
                for srcT, dstG, c0 in ((aT, a_stg, 0),
                                       (bT, b_nm, ch * GCOL)):
                    tT = ab.tile([32, CH], F16, tag="tT")
                    nc.vector.transpose(tT[:], srcT[:])
                    t3 = tT[:].rearrange("q (k f) -> q k f", f=32)
                    for p32 in range(4):
                        nc.vector.tensor_copy(
                            dstG[32 * p32:32 * (p32 + 1),
                                 c0:c0 + GCOL, :],
                            t3[:, p32::4, :])
                # A chunk -> DRAM table rows (row i = p + P*c)
                nc.sync.dma_start(
                    a_loc3[:, ch * GCOL:(ch + 1) * GCOL, :], a_stg[:])

            def phase4_chunk(agg_ap, ch, src, dst):
                """GRU for node chunk ch; returns feat-major ls' tile."""
                o = ch * CH
                agT = io.tile([32, CH], F32, tag="agT")
                tmp2 = io.tile([P, GCOL * 32], F32, tag="tmp2")
                nc.vector.transpose(
                    tmp2[:], agg_ap.rearrange("p a b -> p (a b)"))
                tmp23 = tmp2[:].rearrange("p (c q) -> p c q", q=32)
                agT3 = agT[:].rearrange("f (c w) -> f c w", w=P)
                for p32 in range(4):
                    nc.vector.tensor_copy(
                        agT3[:, :, 32 * p32:32 * (p32 + 1)],
                        tmp23[32 * p32:32 * (p32 + 1), :, :])
                lsc = io.tile([32, CH], F32, tag="lsio")
                nc.sync.dma_start(lsc[:], src[:, o:o + CH])
                pxm = pmm.tile([96, CH], F32, tag="mm", space="PSUM")
                phm = pmm.tile([96, CH], F32, tag="mm", space="PSUM")
                for k in range(CH // NMM):
                    nc.tensor.matmul(pxm[:, k * NMM:(k + 1) * NMM],
                                     w_ap, agT[:, k * NMM:(k + 1) * NMM])
                for k in range(CH // NMM):
                    nc.tensor.matmul(phm[:, k * NMM:(k + 1) * NMM],
                                     u_ap, lsc[:, k * NMM:(k + 1) * NMM])
                # TT requires equal base partitions on both SB inputs;
                # the schedule below realigns operands via ACT placement.
                xm = io.tile([96, CH], F32, tag="xm")
                hm = io.tile([96, CH], F32, tag="hm")
                sc = ab.tile([32, CH], F32, tag="sc")
                nc.scalar.activation(xm[:], pxm[:], AF.Identity, bias=b0_ap)
                nc.scalar.activation(hm[:], phm[:], AF.Identity, bias=b1_ap)
                # t1: xm[0:64] += hm[0:64]  (z and r pre-activations)
                nc.vector.tensor_tensor(xm[0:64, :], xm[0:64, :],
                                        hm[0:64, :], ALU.add)
                # z@xm[0:32], r@xm[32:64]
                nc.scalar.activation(xm[0:64, :], xm[0:64, :], AF.Sigmoid)
                # realign hh to base 32 (hm[0:64] is dead now)
                nc.scalar.activation(hm[32:64, :], hm[64:96, :], AF.Identity)
                # rh = r*hh -> hm[64:96] (base-64 for the t2 add)
                nc.vector.tensor_tensor(hm[64:96, :], xm[32:64, :],
                                        hm[32:64, :], ALU.mult)
                # t2 = xh + rh -> xm[64:96]
                nc.vector.tensor_tensor(xm[64:96, :], xm[64:96, :],
                                        hm[64:96, :], ALU.add)
                # cand -> hm[0:32] (base 0)
                nc.scalar.activation(hm[0:32, :], xm[64:96, :], AF.Tanh)
                # dd = ls - cand -> sc;  e2 = z*dd -> sc
                nc.vector.tensor_tensor(sc[:], lsc[:], hm[0:32, :],
                                        ALU.subtract)
                nc.vector.tensor_tensor(sc[:], xm[0:32, :], sc[:], ALU.mult)
                # ls' = cand + e2 -> lsc
                nc.vector.tensor_tensor(lsc[:], hm[0:32, :], sc[:], ALU.add)
                nc.sync.dma_start(dst[:, o:o + CH], lsc[:])
                return lsc

            # ---- t=0 A/B from ls0 (AG parts fire as chunks complete) ----
            next_q = 0
            for ch in range(n_ch):
                lsc = io.tile([32, CH], F32, tag="lsio")
                nc.sync.dma_start(lsc[:], ls0[:, ch * CH:(ch + 1) * CH])
                phase1_chunk(lsc, ch)
                while next_q < 3 and ch + 1 >= ag_thresh[next_q]:
                    allgather_part(next_q, a_exts[0])
                    next_q += 1
            for q in range(next_q, 4):
                allgather_part(q, a_exts[0])

            for t in range(cfg.T):
                src = ls_src(t)
                dst = ls_dst(t)
                a_ext = a_exts[t % 2]
                ae_next = a_exts[(t + 1) % 2]
                done_ch = 0
                next_q = 0

                for cb in range(NB):
                    cb0 = cb * W
                    # ---- phase 3: gather rounds for this column block ----
                    agg = agp.tile([P, W, 32], F32, tag="agg")
                    nc.vector.tensor_copy(
                        agg[:], corr_sb[:, cb0:cb0 + W, None]
                        .to_broadcast([P, W, 32]))
                    for r in range(R):
                        c_hi = min(C_r[r], cb0 + W)
                        if c_hi <= cb0:
                            break          # C_r is non-increasing
                        cw = c_hi - cb0
                        rb = rp.tile([P, W, 32], F16, tag="rb")
                        rbv = rb[:, 0:cw, :]
                        rbf = rb[:].rearrange("p a b -> p (a b)")
                        nc.vector.tensor_copy(rbv, b_nm[:, cb0:c_hi, :])
                        # indirect gather: one column per instruction (the
                        # HW DGE supports only one offset per partition)
                        for cc in range(cw):
                            col = off_r[r] + cb0 + cc
                            nc.gpsimd.indirect_dma_start(
                                out=rbf[:, cc * 32:(cc + 1) * 32],
                                out_offset=None,
                                in_=a_ext[:, :],
                                in_offset=bass.IndirectOffsetOnAxis(
                                    ap=idx_sb[:, col:col + 1], axis=0),
                                compute_op=ALU.add,
                            )
                        # selu: m=min(x,0); v=LAM*max(x,0) (in-place rb);
                        # e=exp(m); s=LAM*ALPHA*e-LAM*ALPHA; s+=v; agg+=s
                        mt = rp.tile([P, W, 32], F16, tag="mt")
                        mtv = mt[:, 0:cw, :]
                        nc.vector.tensor_scalar(mtv, rbv, 0.0, None, ALU.min)
                        nc.vector.tensor_scalar(rbv, rbv, 0.0, LAM, ALU.max,
                                                ALU.mult)
                        nc.scalar.activation(mtv, mtv, AF.Exp)
                        nc.vector.tensor_scalar(mtv, mtv, LAM * ALPHA,
                                                -LAM * ALPHA, ALU.mult,
                                                ALU.add)
                        nc.vector.tensor_tensor(mtv, mtv, rbv, ALU.add)
                        av = agg[:, 0:cw, :]
                        nc.vector.tensor_tensor(av, av, mtv, ALU.add)

                    # ---- phase 4 (+ next iteration's A/B) per chunk ----
                    for j in range(CPB):
                        ch = cb * CPB + j
                        lsc = phase4_chunk(
                            agg[:, j * GCOL:(j + 1) * GCOL, :], ch, src, dst)
                        if t < cfg.T - 1:
                            phase1_chunk(lsc, ch)
                            done_ch += 1
                            while (next_q < 3
                                   and done_ch >= ag_thresh[next_q]):
                                allgather_part(next_q, ae_next)
                                next_q += 1
                if t < cfg.T - 1:
                    for q in range(next_q, 4):
                        allgather_part(q, ae_next)

    nc.compile()
    return nc


# ---------------------------------------------------------------------------
# Host-side glue
# ---------------------------------------------------------------------------

def _selu_np(x):
    return (LAM * (np.maximum(x, 0.0)
            + ALPHA * (np.expm1(np.minimum(x, 0.0))))).astype(np.float32)


def make_in_maps(cfg, pre, states_action, Wm, bm, W, U, b):
    Wm = np.asarray(Wm, np.float32)
    wpack = np.zeros((96, 260), np.float32)
    wpack[0:32, 0:32] = Wm[:32]
    wpack[0:32, 32:64] = Wm[32:]
    wpack[0:32, 64] = np.asarray(bm, np.float32)
    wpack[0:32, 65:161] = np.asarray(W, np.float32)
    wpack[0:32, 161:257] = np.asarray(U, np.float32)
    wpack[0:96, 257] = np.asarray(b[0], np.float32)
    wpack[0:96, 258] = np.asarray(b[1], np.float32)
    in_maps = []
    for c in range(cfg.NC):
        order = pre["orders"][c]
        ls0 = np.ascontiguousarray(
            np.asarray(states_action, np.float32)[c * cfg.EL + order].T)
        in_maps.append({
            "ls0": ls0,
            "idx": pre["idx"][c],
            "corr": pre["corr"][c],
            "wpack": wpack,
        })
    return in_maps


def readout_host(cfg, pre, ls_outs, states_graph_ids, num_graphs,
                 Wr1, br1, Wr2, br2, Wr3, br3):
    E, EL = cfg.E, cfg.EL
    ls_full = np.empty((E, 32), np.float32)
    for c in range(cfg.NC):
        order = pre["orders"][c]
        ls_full[c * EL + order] = ls_outs[c].T
    gids = np.asarray(states_graph_ids, np.int64)
    gs = np.zeros((num_graphs, 32), np.float32)
    np.add.at(gs, gids, ls_full)
    h = _selu_np(gs @ np.asarray(Wr1, np.float32) + np.asarray(br1, np.float32))
    h = _selu_np(h @ np.asarray(Wr2, np.float32) + np.asarray(br2, np.float32))
    a = np.maximum(h @ np.asarray(Wr3, np.float32)
                   + np.asarray(br3, np.float32), 0.0)
    return a.astype(np.float32)


_BUILD_CACHE = {}


def run_device(cfg, pre, in_maps, use_sim=False, trace=False):
    key = (cfg.E, cfg.T, tuple(pre["C_r"]))
    if key not in _BUILD_CACHE:
        _BUILD_CACHE[key] = build_full(cfg, pre["C_r"], pre["CTOT"])
    nc = _BUILD_CACHE[key]
    if use_sim:
        from concourse.bass_interp import MultiCoreSim
        sim = MultiCoreSim(nc, num_cores=cfg.NC)
        for c in range(cfg.NC):
            for k, v in in_maps[c].items():
                sim.cores[c].tensor(k)[:] = v
        sim.simulate()
        outs = [np.array(sim.cores[c].mem_tensor("ls_out"))
                for c in range(cfg.NC)]
        return outs, None
    res = run_bass_kernel_spmd(nc, in_maps, core_ids=list(range(cfg.NC)),
                               trace=trace)
    outs = [res.results[c]["ls_out"] for c in range(cfg.NC)]
    return outs, res


def _kernel_impl(inputs, use_sim=False, T=8, num_graphs=64, trace=False):
    states_action = np.asarray(inputs["states_action"], np.float32)
    E = states_action.shape[0]
    cfg = Cfg(E=E, T=T, n_cores=8)
    pre = preprocess(cfg, inputs["states_first"], inputs["states_second"])
    in_maps = make_in_maps(cfg, pre, states_action, inputs["Wm"],
                           inputs["bm"], inputs["W"], inputs["U"], inputs["b"])
    ls_outs, res = run_device(cfg, pre, in_maps, use_sim=use_sim, trace=trace)
    a = readout_host(cfg, pre, ls_outs, inputs["states_graph_ids"], num_graphs,
                     inputs["Wr1"], inputs["br1"], inputs["Wr2"],
                     inputs["br2"], inputs["Wr3"], inputs["br3"])
    return a, res


def kernel(**inputs):
    a, _ = _kernel_impl(inputs)
    return a

